# revision 1
# baseline (speedup 1.0000x reference)
"""Trainium2 Bass kernel for nn_Block_86921548136965 (gnn_message_passing).

Transformer block (LN->MHA->LN->MLP) + global neighbor max-pool + BN/GELU +
3-NN inverse-distance interpolation, data-parallel over batch across 8
NeuronCores with an on-device AllGather for the global gather table.

Self-contained: hardcodes shapes/sharding; only needs concourse (bass) + numpy.
"""
import os
import sys

sys.path.insert(0, "/opt/trn_rl_repo")

DEBUG = bool(os.environ.get("BASSK_DEBUG"))

import numpy as np
import ml_dtypes

import concourse.bass as bass
import concourse.bacc as bacc
import concourse.tile as tile
from concourse import mybir
from concourse import bass_utils
from concourse.masks import make_identity

# problem shapes
B, G, C, H = 64, 512, 384, 6
HD = C // H  # 64
N2, K = 128, 32
HID = 4 * C  # 1536
NCORES = 8
BL = B // NCORES  # 8 batches per core
ROWS = B * G  # 32768 global rows

F32 = mybir.dt.float32
F32R = mybir.dt.float32r
BF16 = mybir.dt.bfloat16
I32 = mybir.dt.int32
AX = mybir.AxisListType
OP = mybir.AluOpType
AF = mybir.ActivationFunctionType

BIG = 1.0e30
EPS_LN = 1e-5
EPS_BN = 1e-5
EPS_W = 1e-8

_CACHE = {}


def _build_program():
    nc = bacc.Bacc("TRN2", target_bir_lowering=False, debug=False,
                   num_devices=NCORES)

    # ---------------- DRAM I/O ----------------
    d_x = nc.dram_tensor("x_in", [BL, G, C], F32, kind="ExternalInput")
    d_wqk = nc.dram_tensor("wqkT", [C, 2 * C], BF16, kind="ExternalInput")
    d_qkb = nc.dram_tensor("qk_bias", [128, 6], F32, kind="ExternalInput")
    d_wv = nc.dram_tensor("wvT", [C, C], BF16, kind="ExternalInput")
    d_vbr = nc.dram_tensor("v_bias_rep", [128, C], F32, kind="ExternalInput")
    d_pjT = nc.dram_tensor("projT", [C, C], BF16, kind="ExternalInput")
    d_pbr = nc.dram_tensor("proj_b_rep", [128, C], F32, kind="ExternalInput")
    d_w1 = nc.dram_tensor("wfc1T", [C, HID], BF16, kind="ExternalInput")
    d_f1b = nc.dram_tensor("fc1_bias", [128, 12], F32, kind="ExternalInput")
    d_w2 = nc.dram_tensor("wfc2T", [HID, C], BF16, kind="ExternalInput")
    d_f2br = nc.dram_tensor("fc2_b_rep", [128, C], F32, kind="ExternalInput")
    d_bns = nc.dram_tensor("bn_scale_rep", [128, C], F32, kind="ExternalInput")
    d_bnh = nc.dram_tensor("bn_shift_rep", [128, C], F32, kind="ExternalInput")
    d_rep6 = nc.dram_tensor("rep6", [6, C], F32, kind="ExternalInput")
    d_nidx = nc.dram_tensor("nidx", [128, BL * K], I32, kind="ExternalInput")
    d_cidx = nc.dram_tensor("cidx", [128, BL], I32, kind="ExternalInput")
    d_l1a = nc.dram_tensor("l1aug", [BL, 5, G], F32, kind="ExternalInput")
    d_l2a = nc.dram_tensor("l2aug", [BL, 5, N2], F32, kind="ExternalInput")
    d_out = nc.dram_tensor("out", [BL, G, C], F32, kind="ExternalOutput")
    if DEBUG:
        d_dxn = nc.dram_tensor("dbg_xn", [BL, 128, 4, C], F32,
                               kind="ExternalOutput")
        d_dqkT = nc.dram_tensor("dbg_qkT", [BL, 128, 6, G], F32,
                                kind="ExternalOutput")
        d_dva = nc.dram_tensor("dbg_va", [BL, 128, 4, 6, 65], F32,
                               kind="ExternalOutput")
        d_dE = nc.dram_tensor("dbg_E", [BL, 128, 4, G], F32,
                              kind="ExternalOutput")
        d_dx1 = nc.dram_tensor("dbg_x1", [BL, 128, 4, C], F32,
                               kind="ExternalOutput")
        d_dx2 = nc.dram_tensor("dbg_x2", [BL, G, C], F32, kind="ExternalOutput")
        d_dvis = nc.dram_tensor("dbg_vis", [BL, 128, C], F32,
                                kind="ExternalOutput")
        d_dw = nc.dram_tensor("dbg_w", [BL, 4, 128, N2], F32,
                              kind="ExternalOutput")
        d_dsum = nc.dram_tensor("dbg_sum", [BL, 6, G], F32,
                                kind="ExternalOutput")
        d_dpool = nc.dram_tensor("dbg_pool", [BL, 128, C], F32,
                                 kind="ExternalOutput")

    from contextlib import ExitStack
    with tile.TileContext(nc) as tc:
        with tc.tile_pool(name="cpool", bufs=1) as cp, \
             tc.tile_pool(name="dram", bufs=1, space="DRAM") as dp:
            stk = ExitStack()
            wp = stk.enter_context(tc.tile_pool(name="wpool", bufs=1))
            wk = stk.enter_context(tc.tile_pool(name="work", bufs=2))
            psp = stk.enter_context(tc.tile_pool(name="ps", bufs=2, space="PSUM"))

            # ---------------- static loads ----------------
            wqk_s = wp.tile([128, 3, 2 * C], BF16)
            nc.sync.dma_start(wqk_s[:], d_wqk.ap().rearrange(
                "(cc p) f -> p cc f", p=128))
            wv_s = wp.tile([128, 3, C], BF16)
            nc.sync.dma_start(wv_s[:], d_wv.ap().rearrange(
                "(cc p) f -> p cc f", p=128))
            pjT_s = wp.tile([128, 3, C], BF16)
            nc.sync.dma_start(pjT_s[:], d_pjT.ap().rearrange(
                "(cc p) f -> p cc f", p=128))
            w1_s = wp.tile([128, 3, HID], BF16)
            nc.sync.dma_start(w1_s[:], d_w1.ap().rearrange(
                "(cc p) f -> p cc f", p=128))
            w2_s = wp.tile([128, 12, C], BF16)
            nc.sync.dma_start(w2_s[:], d_w2.ap().rearrange(
                "(cc p) f -> p cc f", p=128))
            rep6_s = wp.tile([6, C], F32R)
            nc.sync.dma_start(rep6_s[:], d_rep6.ap().bitcast(F32R))

            qkb_s = wp.tile([128, 6], F32)
            nc.sync.dma_start(qkb_s[:], d_qkb.ap())
            f1b_s = wp.tile([128, 12], F32)
            nc.sync.dma_start(f1b_s[:], d_f1b.ap())
            vbr_s = wp.tile([128, C], F32)
            nc.sync.dma_start(vbr_s[:], d_vbr.ap())
            pbr_s = wp.tile([128, C], F32)
            nc.sync.dma_start(pbr_s[:], d_pbr.ap())
            f2br_s = wp.tile([128, C], F32)
            nc.sync.dma_start(f2br_s[:], d_f2br.ap())
            bns_s = cp.tile([128, C], F32)
            nc.sync.dma_start(bns_s[:], d_bns.ap())
            bnh_s = cp.tile([128, C], F32)
            nc.sync.dma_start(bnh_s[:], d_bnh.ap())
            nidx_s = cp.tile([128, BL * K], I32)
            nc.sync.dma_start(nidx_s[:], d_nidx.ap())
            cidx_s = cp.tile([128, BL], I32)
            nc.sync.dma_start(cidx_s[:], d_cidx.ap())
            l1a_s = cp.tile([5, BL, G], F32)
            nc.sync.dma_start(l1a_s[:], d_l1a.ap().rearrange("b r s -> r b s"))
            l2a_s = cp.tile([5, BL, N2], F32)
            nc.sync.dma_start(l2a_s[:], d_l2a.ap().rearrange("b r s -> r b s"))

            ident = cp.tile([128, 128], F32)
            make_identity(nc, ident[:])
            eps_s = cp.tile([128, 1], F32)
            nc.vector.memset(eps_s[:], EPS_LN)

            # internal DRAM
            sums_d = dp.tile([6, G], F32)
            ag_in = dp.tile([BL * G, C], BF16)
            table = dp.tile([ROWS, C], BF16)
            x2d = dp.tile([BL * G, C], F32)

            # =================== PHASE A: transformer ===================
            HBL = BL // 2
            for b in range(BL):
                xr = wk.tile([128, 4, C], F32, name=f"xr{b}", tag="xr")
                nc.sync.dma_start(xr[:], d_x.ap()[b].rearrange(
                    "(ch p) c -> p ch c", p=128))
                # residual base: x + proj_b
                xb = wk.tile([128, 4, C], F32, name=f"xb{b}", tag="xb", bufs=2)
                pb_b = bass.AP(pbr_s.tensor, pbr_s[:].offset,
                               [pbr_s[:].ap[0], [0, 4], pbr_s[:].ap[1]])
                nc.vector.tensor_tensor(out=xb[:], in0=xr[:], in1=pb_b,
                                        op=OP.add)

                # ---- LN1 -> xn (normalized, no affine; affine folded) ----
                xn = wk.tile([128, 4, C], BF16, name=f"xn{b}", tag="xn", bufs=2)
                for ch in range(4):
                    st6 = wk.tile([128, 6], F32, name=f"st{b}{ch}", tag="st")
                    nc.vector.bn_stats(out=st6[:], in_=xr[:, ch, :])
                    mv = wk.tile([128, 2], F32, name=f"mv{b}{ch}", tag="mv")
                    nc.vector.bn_aggr(out=mv[:], in_=st6[:])
                    sd = wk.tile([128, 1], F32, name=f"sd{b}{ch}", tag="sd")
                    nc.scalar.activation(sd[:], mv[:, 1:2], AF.Sqrt, bias=eps_s[:])
                    rs = wk.tile([128, 1], F32, name=f"rg{b}{ch}", tag="rg")
                    nc.vector.reciprocal(rs[:], sd[:])
                    nc.vector.tensor_scalar(out=xn[:, ch, :], in0=xr[:, ch, :],
                                            scalar1=mv[:, 0:1], scalar2=rs[:],
                                            op0=OP.subtract, op1=OP.mult)
                if DEBUG:
                    nc.sync.dma_start(d_dxn.ap()[b], xn[:])
                # ---- transpose xn -> xnT [c, s] (HWDGE xbar, 3D out) ----
                xnT = wk.tile([128, 3, G], BF16, name=f"xnT{b}", tag="xnT", bufs=2)
                for ch in range(4):
                    eng = nc.sync if ch % 2 == 0 else nc.scalar
                    eng.dma_start_transpose(
                        xnT[:, :, ch * 128:(ch + 1) * 128], xn[:, ch, :])

                # ---- qkT = Weff_qk @ xnT + bias ----
                qkT = wk.tile([128, 6, G], BF16, name=f"qkT{b}", tag="qkT", bufs=2)
                for f in range(6):
                    ps1 = psp.tile([128, G], F32, name=f"qk{b}{f}", tag="ps_a")
                    for cc in range(3):
                        nc.tensor.matmul(ps1[:],
                                         wqk_s[:, cc, f * 128:(f + 1) * 128],
                                         xnT[:, cc, :],
                                         start=(cc == 0), stop=(cc == 2))
                    nc.scalar.activation(qkT[:, f, :], ps1[:], AF.Identity,
                                         bias=qkb_s[:, f:f + 1])

                if DEBUG:
                    pass  # dbg_qkT disabled under bf16
                # ---- v = xn @ WvT + bias, stored as vaug [s, h, 65] ----
                vaug = wk.tile([128, 4, 6, 65], BF16, name=f"va{b}", tag="va",
                               bufs=2)
                nc.gpsimd.memset(vaug[:], 1.0)
                for sch in range(4):
                    ps2 = psp.tile([128, C], F32, name=f"v{b}{sch}", tag="ps_b")
                    for cc in range(3):
                        nc.tensor.matmul(ps2[:],
                                         xnT[:, cc, sch * 128:(sch + 1) * 128],
                                         wv_s[:, cc, :],
                                         start=(cc == 0), stop=(cc == 2))
                    nc.vector.tensor_tensor(
                        out=vaug[:, sch, :, 0:64],
                        in0=ps2[:].rearrange("p (h d) -> p h d", h=6),
                        in1=vbr_s[:].rearrange("p (h d) -> p h d", h=6),
                        op=OP.add)

                if DEBUG:
                    pass  # dbg_va disabled under bf16
                # ---- attention per head ----
                oTr = wk.tile([128, 3, G], F32, name=f"oTr{b}", tag="oTr", bufs=1)
                sums = wk.tile([1, 6, G], F32, name=f"sm{b}", tag="sm",
                               bufs=2)
                for h in range(6):
                    po = (h % 2) * 64
                    qT = qkT[po:po + 64, h // 2, :]
                    kT = qkT[po:po + 64, 3 + h // 2, :]
                    Eh = wk.tile([128, 4, G], BF16, name=f"E{b}{h}", tag="E", bufs=2)
                    for kc in range(4):
                        ps3 = psp.tile([128, G], F32, name=f"s{b}{h}{kc}",
                                       tag="ps_a")
                        nc.tensor.matmul(ps3[:],
                                         kT[:, kc * 128:(kc + 1) * 128],
                                         qT, start=True, stop=True)
                        nc.scalar.activation(Eh[:, kc, :], ps3[:], AF.Exp)
                    pass
                    ps4 = psp.tile([65, G], F32, name=f"o{b}{h}", tag="ps_c")
                    for kc in range(4):
                        nc.tensor.matmul(ps4[:], vaug[:, kc, h, :],
                                         Eh[:, kc, :],
                                         start=(kc == 0), stop=(kc == 3))
                    nc.scalar.copy(oTr[po:po + 64, h // 2, :], ps4[0:64, :])
                    nc.scalar.copy(sums[0:1, h, :], ps4[64:65, :])

                # ---- normalization matrix R, scale oT ----
                nc.sync.dma_start(
                    sums_d[:, :].rearrange("h q -> (h q)"), sums[0:1, :, :])
                if DEBUG:
                    nc.sync.dma_start(
                        d_dsum.ap()[b].rearrange("h q -> (h q)"),
                        sums[0:1, :, :])
                sums6 = wk.tile([6, G], F32, name=f"s6{b}", tag="s6", bufs=2)
                nc.sync.dma_start(sums6[:], sums_d[:, :])
                srec = wk.tile([6, G], F32R, name=f"sr{b}", tag="sr", bufs=2)
                with nc.allow_low_precision("fp32r is fp32-width"):
                    nc.vector.reciprocal(srec[:], sums6[:])
                oTs = wk.tile([128, 3, G], BF16, name=f"oTs{b}", tag="oTs", bufs=2)
                for cc in range(3):
                    ps5 = psp.tile([128, G], F32, name=f"R{b}{cc}", tag="ps_a")
                    nc.tensor.matmul(ps5[:], rep6_s[:, cc * 128:(cc + 1) * 128],
                                     srec[:], start=True, stop=True)
                    nc.vector.tensor_tensor(out=oTs[:, cc, :],
                                            in0=oTr[:, cc, :], in1=ps5[:],
                                            op=OP.mult)

                # ---- proj + residual -> x1 ----
                x1 = wk.tile([128, 4, C], F32, name=f"x1{b}", tag="x1", bufs=2)
                for sch in range(4):
                    ps6 = psp.tile([128, C], F32, name=f"pj{b}{sch}",
                                   tag="ps_b")
                    for cc in range(3):
                        nc.tensor.matmul(ps6[:],
                                         oTs[:, cc, sch * 128:(sch + 1) * 128],
                                         pjT_s[:, cc, :],
                                         start=(cc == 0), stop=(cc == 2))
                    nc.vector.tensor_tensor(out=x1[:, sch, :], in0=ps6[:],
                                            in1=xb[:, sch, :], op=OP.add)

                if DEBUG:
                    nc.sync.dma_start(d_dx1.ap()[b], x1[:])
                # ---- LN2 -> xn2 ----
                xn2 = wk.tile([128, 4, C], BF16, name=f"xn2{b}", tag="xn", bufs=2)
                for ch in range(4):
                    st6b = wk.tile([128, 6], F32, name=f"su{b}{ch}", tag="st")
                    nc.vector.bn_stats(out=st6b[:], in_=x1[:, ch, :])
                    mvb = wk.tile([128, 2], F32, name=f"mw{b}{ch}", tag="mv")
                    nc.vector.bn_aggr(out=mvb[:], in_=st6b[:])
                    sdb = wk.tile([128, 1], F32, name=f"se{b}{ch}", tag="sd")
                    nc.scalar.activation(sdb[:], mvb[:, 1:2], AF.Sqrt,
                                         bias=eps_s[:])
                    rsb = wk.tile([128, 1], F32, name=f"rh{b}{ch}", tag="rg")
                    nc.vector.reciprocal(rsb[:], sdb[:])
                    nc.vector.tensor_scalar(out=xn2[:, ch, :], in0=x1[:, ch, :],
                                            scalar1=mvb[:, 0:1], scalar2=rsb[:],
                                            op0=OP.subtract, op1=OP.mult)
                xn2T = wk.tile([128, 3, G], BF16, name=f"x2T{b}", tag="xnT", bufs=2)
                for ch in range(4):
                    eng = nc.sync if ch % 2 == 0 else nc.scalar
                    eng.dma_start_transpose(
                        xn2T[:, :, ch * 128:(ch + 1) * 128], xn2[:, ch, :])

                # ---- fc1 + gelu -> uT ----
                uT = wk.tile([128, 12, G], BF16, name=f"uT{b}", tag="uT", bufs=2)
                for f in range(12):
                    ps7 = psp.tile([128, G], F32, name=f"f1{b}{f}", tag="ps_a")
                    for cc in range(3):
                        nc.tensor.matmul(ps7[:],
                                         w1_s[:, cc, f * 128:(f + 1) * 128],
                                         xn2T[:, cc, :],
                                         start=(cc == 0), stop=(cc == 2))
                    nc.scalar.activation(uT[:, f, :], ps7[:], AF.Gelu,
                                         bias=f1b_s[:, f:f + 1])

                # ---- fc2 + residual -> x2; dump fp32 + bf16 ----
                for sch in range(4):
                    ps8 = psp.tile([128, C], F32, name=f"f2{b}{sch}",
                                   tag="ps_b")
                    for f in range(12):
                        nc.tensor.matmul(ps8[:],
                                         uT[:, f, sch * 128:(sch + 1) * 128],
                                         w2_s[:, f, :],
                                         start=(f == 0), stop=(f == 11))
                    x2c = wk.tile([128, C], F32, name=f"x2{b}{sch}", tag="x2c", bufs=2)
                    nc.vector.tensor_tensor(out=x2c[:], in0=ps8[:],
                                            in1=x1[:, sch, :], op=OP.add)
                    x2f = wk.tile([128, C], F32, name=f"x2f{b}{sch}",
                                  tag="x2f")
                    nc.vector.tensor_tensor(out=x2f[:], in0=x2c[:],
                                            in1=f2br_s[:], op=OP.add)
                    x2b = wk.tile([128, C], BF16, name=f"x2b{b}{sch}",
                                  tag="x2b")
                    nc.vector.tensor_copy(x2b[:], x2f[:])
                    row0 = b * G + sch * 128
                    nc.sync.dma_start(x2d[row0:row0 + 128, :], x2f[:])
                    nc.sync.dma_start(ag_in[row0:row0 + 128, :], x2b[:])
                if b == HBL - 1:
                    # first-half AllGather overlaps batches HBL..BL-1
                    nc.gpsimd.collective_compute(
                        "AllGather", OP.bypass,
                        replica_groups=[list(range(NCORES))],
                        ins=[ag_in[0:HBL * G, :]],
                        outs=[table[0:NCORES * HBL * G, :]])
                    if DEBUG:
                        nc.sync.dma_start(
                            d_dx2.ap()[b, sch * 128:(sch + 1) * 128, :],
                            x2f[:])

            # =================== PHASE B: second-half AllGather ==========
            stk.close()
            stk2 = ExitStack()
            gp = stk2.enter_context(tc.tile_pool(name="gat", bufs=2))
            psp = stk2.enter_context(
                tc.tile_pool(name="psC", bufs=2, space="PSUM"))
            nc.gpsimd.collective_compute(
                "AllGather", OP.bypass,
                replica_groups=[list(range(NCORES))],
                ins=[ag_in[HBL * G:BL * G, :]],
                outs=[table[NCORES * HBL * G:ROWS, :]])

            # =================== PHASE C: gather/pool/3NN ===================
            cent = gp.tile([128, BL, C], BF16, name="cent", tag="cent", bufs=1)
            for b in range(BL):
                nc.gpsimd.indirect_dma_start(
                    out=cent[:, b, :], out_offset=None,
                    in_=table.opt(),
                    in_offset=bass.IndirectOffsetOnAxis(
                        ap=cidx_s[:, b:b + 1], axis=0))
            for b in range(BL):
                acc = gp.tile([128, K, C], BF16, name=f"acc{b}", tag="acc")
                for k in range(K):
                    nc.gpsimd.indirect_dma_start(
                        out=acc[:, k, :], out_offset=None,
                        in_=table.opt(),
                        in_offset=bass.IndirectOffsetOnAxis(
                            ap=nidx_s[:, b * K + k:b * K + k + 1], axis=0))
                # max-pool tree over K (in-place halving inside acc)
                for half in (16, 8, 4, 2, 1):
                    nc.vector.tensor_tensor(
                        out=acc[:, 0:half, :], in0=acc[:, 0:half, :],
                        in1=acc[:, half:2 * half, :], op=OP.max)
                pool1 = acc
                # BN (x2 & affine folded) + gelu + 0.3*centers
                pb1 = gp.tile([128, C], F32, name=f"pb1{b}", tag="pb1")
                nc.vector.tensor_tensor(out=pb1[:], in0=pool1[:, 0, :], in1=bns_s[:],
                                        op=OP.mult)
                pb2 = gp.tile([128, C], F32, name=f"pb2{b}", tag="pb2")
                nc.vector.tensor_tensor(out=pb2[:], in0=pb1[:], in1=bnh_s[:],
                                        op=OP.add)
                gl = gp.tile([128, C], F32, name=f"gl{b}", tag="gl")
                nc.scalar.activation(gl[:], pb2[:], AF.Gelu)
                vis = gp.tile([128, C], F32R, name=f"vis{b}", tag="vis")
                nc.vector.scalar_tensor_tensor(out=vis[:], in0=cent[:, b, :],
                                               scalar=0.3, in1=gl[:],
                                               op0=OP.mult, op1=OP.add)
                if DEBUG:
                    nc.sync.dma_start(d_dvis.ap()[b],
                                      vis[:].bitcast(F32))
                    nc.sync.dma_start(d_dpool.ap()[b], pb2[:])

                for ch in range(4):
                    psd = psp.tile([128, N2], F32, name=f"d2{b}{ch}",
                                   tag="ps_tr")
                    nc.tensor.matmul(psd[:],
                                     l1a_s[:, b, ch * 128:(ch + 1) * 128],
                                     l2a_s[:, b, :], start=True, stop=True)
                    d2s = gp.tile([128, N2], F32, name=f"d2s{b}{ch}", tag="d2s")
                    nc.vector.tensor_copy(d2s[:], psd[:])
                    m1 = gp.tile([128, 1], F32, name=f"m1{b}{ch}", tag="m1")
                    nc.vector.tensor_reduce(out=m1[:], in_=d2s[:], axis=AX.X,
                                            op=OP.min)
                    msk1 = gp.tile([128, N2], F32, name=f"k1{b}{ch}", tag="k1")
                    nc.vector.tensor_scalar(out=msk1[:], in0=d2s[:],
                                            scalar1=m1[:], scalar2=BIG,
                                            op0=OP.is_le, op1=OP.mult)
                    d2a = gp.tile([128, N2], F32, name=f"da{b}{ch}", tag="da")
                    nc.vector.tensor_tensor(out=d2a[:], in0=d2s[:],
                                            in1=msk1[:], op=OP.add)
                    m2 = gp.tile([128, 1], F32, name=f"m2{b}{ch}", tag="m2")
                    nc.vector.tensor_reduce(out=m2[:], in_=d2a[:], axis=AX.X,
                                            op=OP.min)
                    msk2 = gp.tile([128, N2], F32, name=f"k2{b}{ch}", tag="k2")
                    nc.vector.tensor_scalar(out=msk2[:], in0=d2a[:],
                                            scalar1=m2[:], scalar2=BIG,
                                            op0=OP.is_le, op1=OP.mult)
                    d2b = gp.tile([128, N2], F32, name=f"db{b}{ch}", tag="db")
                    nc.vector.tensor_tensor(out=d2b[:], in0=d2a[:],
                                            in1=msk2[:], op=OP.add)
                    m3 = gp.tile([128, 1], F32, name=f"m3{b}{ch}", tag="m3")
                    nc.vector.tensor_reduce(out=m3[:], in_=d2b[:], axis=AX.X,
                                            op=OP.min)
                    msk = gp.tile([128, N2], F32, name=f"kk{b}{ch}", tag="kk")
                    nc.vector.tensor_scalar(out=msk[:], in0=d2s[:],
                                            scalar1=m3[:], scalar2=None,
                                            op0=OP.is_le)
                    d2e = gp.tile([128, N2], F32, name=f"de{b}{ch}", tag="de")
                    nc.vector.tensor_scalar(out=d2e[:], in0=d2s[:],
                                            scalar1=EPS_W, scalar2=None,
                                            op0=OP.add)
                    wiv = gp.tile([128, N2], F32, name=f"wi{b}{ch}", tag="wi")
                    nc.vector.reciprocal(wiv[:], d2e[:])
                    w0 = gp.tile([128, N2], F32, name=f"w0{b}{ch}", tag="w0")
                    nc.vector.tensor_tensor(out=w0[:], in0=msk[:], in1=wiv[:],
                                            op=OP.mult)
                    ssum = gp.tile([128, 1], F32, name=f"ss{b}{ch}", tag="ss")
                    nc.vector.tensor_reduce(out=ssum[:], in_=w0[:], axis=AX.X,
                                            op=OP.add)
                    rsm = gp.tile([128, 1], F32, name=f"rm{b}{ch}", tag="rm")
                    nc.vector.reciprocal(rsm[:], ssum[:])
                    wfin = gp.tile([128, N2], F32, name=f"wf{b}{ch}", tag="wf")
                    nc.vector.tensor_scalar(out=wfin[:], in0=w0[:],
                                            scalar1=rsm[:], scalar2=None,
                                            op0=OP.mult)
                    if DEBUG:
                        nc.sync.dma_start(d_dw.ap()[b, ch], wfin[:])
                    # transpose W -> [j, p]
                    pst = psp.tile([128, N2], F32, name=f"wt{b}{ch}",
                                   tag="ps_tr")
                    nc.tensor.transpose(pst[:], wfin[:], ident[:])
                    wts = gp.tile([128, N2], F32R, name=f"wr{b}{ch}", tag="wr")
                    nc.vector.tensor_copy(wts[:], pst[:])
                    # interp = W @ vis ; add x2
                    psi = psp.tile([128, C], F32, name=f"ip{b}{ch}", tag="ps_b")
                    nc.tensor.matmul(psi[:], wts[:], vis[:], start=True,
                                     stop=True)
                    x2r = gp.tile([128, C], F32, name=f"x2r{b}{ch}", tag="x2r")
                    row0 = b * G + ch * 128
                    nc.sync.dma_start(x2r[:], x2d[row0:row0 + 128, :])
                    oc = gp.tile([128, C], F32, name=f"oc{b}{ch}", tag="oc")
                    nc.vector.tensor_tensor(out=oc[:], in0=psi[:], in1=x2r[:],
                                            op=OP.add)
                    nc.sync.dma_start(
                        d_out.ap()[b, ch * 128:(ch + 1) * 128, :], oc[:])
            stk2.close()

    nc.compile()
    return nc


def _prep_inputs(x, level1_center, level2_center, ln1_g, ln1_b, qkv_w, proj_w,
                 proj_b, ln2_g, ln2_b, fc1_w, fc1_b, fc2_w, fc2_b, bn_g, bn_b,
                 bn_mean, bn_var, level1_index, level2_index):
    """Build the per-core in_maps (host-side folding + sharding)."""
    f32 = np.float32
    x = np.ascontiguousarray(np.asarray(x, f32))
    l1c = np.asarray(level1_center, f32)
    l2c = np.asarray(level2_center, f32)
    ln1_g = np.asarray(ln1_g, f32); ln1_b = np.asarray(ln1_b, f32)
    ln2_g = np.asarray(ln2_g, f32); ln2_b = np.asarray(ln2_b, f32)
    qkv_w = np.asarray(qkv_w, f32); proj_w = np.asarray(proj_w, f32)
    proj_b = np.asarray(proj_b, f32)
    fc1_w = np.asarray(fc1_w, f32); fc1_b = np.asarray(fc1_b, f32)
    fc2_w = np.asarray(fc2_w, f32); fc2_b = np.asarray(fc2_b, f32)
    bn_g = np.asarray(bn_g, f32); bn_b = np.asarray(bn_b, f32)
    bn_mean = np.asarray(bn_mean, f32); bn_var = np.asarray(bn_var, f32)
    l1i = np.asarray(level1_index).astype(np.int64).reshape(B, N2, K)
    l2i = np.asarray(level2_index).astype(np.int64).reshape(B, N2)

    # remap global row ids to the split-AllGather table layout:
    # table[0:16384) = concat_c concat_{b<4} batch rows; upper half b>=4
    def _remap(r):
        c = r // (BL * G)
        rem = r % (BL * G)
        b = rem // G
        g = rem % G
        half = (b >= BL // 2).astype(np.int64)
        bb = b - half * (BL // 2)
        return (half * (B * G // 2) + c * (BL // 2 * G) + bb * G + g)

    l1i = _remap(l1i)
    l2i = _remap(l2i)

    s = HD ** -0.5
    weff = qkv_w * ln1_g[None, :]
    beff = qkv_w @ ln1_b
    weff[:C] *= s
    beff[:C] *= s
    wqkT = np.ascontiguousarray(weff[:2 * C].T.astype(ml_dtypes.bfloat16))
    qk_bias = np.ascontiguousarray(beff[:2 * C].reshape(6, 128).T)
    wvT = np.ascontiguousarray(weff[2 * C:].T.astype(ml_dtypes.bfloat16))
    v_bias_rep = np.ascontiguousarray(
        np.broadcast_to(beff[2 * C:], (128, C)))
    projT = np.ascontiguousarray(proj_w.T.astype(ml_dtypes.bfloat16))
    proj_b_rep = np.ascontiguousarray(np.broadcast_to(proj_b, (128, C)))
    w1eff = fc1_w * ln2_g[None, :]
    f1bias = fc1_b + fc1_w @ ln2_b
    wfc1T = np.ascontiguousarray(w1eff.T.astype(ml_dtypes.bfloat16))
    fc1_bias = np.ascontiguousarray(f1bias.reshape(12, 128).T)
    wfc2T = np.ascontiguousarray(fc2_w.T.astype(ml_dtypes.bfloat16))
    fc2_b_rep = np.ascontiguousarray(np.broadcast_to(fc2_b, (128, C)))
    gs = bn_g / np.sqrt(bn_var + EPS_BN)
    bn_scale_rep = np.ascontiguousarray(
        np.broadcast_to((2.0 * gs).astype(f32), (128, C)))
    bn_shift_rep = np.ascontiguousarray(
        np.broadcast_to((bn_b - bn_mean * gs).astype(f32), (128, C)))
    rep6 = np.zeros((6, C), f32)
    for h in range(H):
        rep6[h, h * HD:(h + 1) * HD] = 1.0

    # 3NN augmented coordinate blocks
    # d2[p, j] = l1.(-2 l2) + |l2|^2 + |l1|^2
    l1n = (l1c ** 2).sum(-1)                                 # [B, G]
    l2n = (l2c ** 2).sum(-1)                                 # [B, N2]
    l1aug = np.empty((B, 5, G), f32)
    l1aug[:, 0:3] = np.transpose(l1c, (0, 2, 1))
    l1aug[:, 3] = 1.0
    l1aug[:, 4] = l1n
    l2aug = np.empty((B, 5, N2), f32)
    l2aug[:, 0:3] = -2.0 * np.transpose(l2c, (0, 2, 1))
    l2aug[:, 3] = l2n
    l2aug[:, 4] = 1.0

    shared = {
        "wqkT": wqkT, "qk_bias": qk_bias, "wvT": wvT,
        "v_bias_rep": v_bias_rep, "projT": projT,
        "proj_b_rep": proj_b_rep, "wfc1T": wfc1T, "fc1_bias": fc1_bias,
        "wfc2T": wfc2T, "fc2_b_rep": fc2_b_rep,
        "bn_scale_rep": bn_scale_rep, "bn_shift_rep": bn_shift_rep,
        "rep6": rep6,
    }
    in_maps = []
    for c in range(NCORES):
        b0 = c * BL
        # nidx: [128, BL*K], col b*K+k = l1i[b0+b, p, k]
        nid = np.ascontiguousarray(
            np.transpose(l1i[b0:b0 + BL], (1, 0, 2)).reshape(128, BL * K)
            .astype(np.int32))
        cid = np.ascontiguousarray(l2i[b0:b0 + BL].T.astype(np.int32))
        m = dict(shared)
        m["x_in"] = np.ascontiguousarray(x[b0:b0 + BL])
        m["nidx"] = nid
        m["cidx"] = cid
        m["l1aug"] = np.ascontiguousarray(l1aug[b0:b0 + BL])
        m["l2aug"] = np.ascontiguousarray(l2aug[b0:b0 + BL])
        in_maps.append(m)
    return in_maps


def get_program():
    if "nc" not in _CACHE:
        _CACHE["nc"] = _build_program()
    return _CACHE["nc"]


def run(in_maps, **kw):
    nc = get_program()
    return bass_utils.run_bass_kernel_spmd(
        nc, in_maps, core_ids=list(range(NCORES)), **kw)


def kernel(**inputs):
    in_maps = _prep_inputs(**inputs)
    res = run(in_maps)
    out = np.concatenate([res.results[c]["out"] for c in range(NCORES)],
                         axis=0)
    return out.astype(np.float32)


if __name__ == "__main__":
    np.random.seed(0)
    get_program()
    print("program built + compiled OK")



# revision 2
# speedup vs baseline: 1.0087x; 1.0087x over previous
"""Trainium2 Bass kernel for nn_Block_86921548136965 (gnn_message_passing), v2.

Transformer block (LN->MHA->LN->MLP) + global neighbor max-pool + BN/GELU +
3-NN inverse-distance interpolation, data-parallel over batch across 8
NeuronCores.

v2 changes vs baseline:
- Phase C neighbor gather via one bulk dma_gather per batch (vs 33 SWDGE
  indirect DMAs each) - removes the ~360us serial gpsimd bottleneck.
- 4-chunk AllGather (after local batches 1,3,5,7) so only the last ~3MB
  chunk is exposed.
- 3NN interpolation weights precomputed during Phase A on the otherwise
  idle gpsimd/vector engines.
- x2 kept in SBUF as bf16 (no fp32 DRAM round trip).
- softmax-sum reshape via a single SBUF->SBUF DMA (no DRAM bounce).
- rowsum of interp weights folded into the interp matmul (ones column).
"""
import os
import sys

sys.path.insert(0, "/opt/trn_rl_repo")

import numpy as np
import ml_dtypes

import concourse.bass as bass
import concourse.bacc as bacc
import concourse.tile as tile
from concourse import mybir
from concourse import bass_utils
from concourse.masks import make_identity

# problem shapes
B, G, C, H = 64, 512, 384, 6
HD = C // H  # 64
N2, K = 128, 32
HID = 4 * C  # 1536
NCORES = 8
BL = B // NCORES  # 8 batches per core
ROWS = B * G  # 32768 global rows
CHB = 2  # batches per AllGather chunk
NCHUNK = BL // CHB  # 4
CHROWS = ROWS // NCHUNK  # 8192 rows per chunk
NI = (K + 1) * 128  # 4224 gathered rows per batch (32 neigh + 1 center)
ICOLS = NI // 16  # 264 int16 per partition (16-wrap)

F32 = mybir.dt.float32
F32R = mybir.dt.float32r
BF16 = mybir.dt.bfloat16
I16 = mybir.dt.int16
AX = mybir.AxisListType
OP = mybir.AluOpType
AF = mybir.ActivationFunctionType

BIG = 1.0e30
EPS_LN = 1e-5
EPS_BN = 1e-5
EPS_W = 1e-8

_CACHE = {}


def _build_program():
    nc = bacc.Bacc("TRN2", target_bir_lowering=False, debug=False,
                   num_devices=NCORES)

    # ---------------- DRAM I/O ----------------
    d_x = nc.dram_tensor("x_in", [BL, G, C], F32, kind="ExternalInput")
    d_wqk = nc.dram_tensor("wqkT", [C, 2 * C], BF16, kind="ExternalInput")
    d_qkb = nc.dram_tensor("qk_bias", [128, 6], F32, kind="ExternalInput")
    d_wv = nc.dram_tensor("wvT", [C, C], BF16, kind="ExternalInput")
    d_vbr = nc.dram_tensor("v_bias_rep", [128, C], F32, kind="ExternalInput")
    d_pjT = nc.dram_tensor("projT", [C, C], BF16, kind="ExternalInput")
    d_pbr = nc.dram_tensor("proj_b_rep", [128, C], F32, kind="ExternalInput")
    d_w1 = nc.dram_tensor("wfc1T", [C, HID], BF16, kind="ExternalInput")
    d_f1b = nc.dram_tensor("fc1_bias", [128, 12], F32, kind="ExternalInput")
    d_w2 = nc.dram_tensor("wfc2T", [HID, C], BF16, kind="ExternalInput")
    d_f2br = nc.dram_tensor("fc2_b_rep", [128, C], F32, kind="ExternalInput")
    d_bns = nc.dram_tensor("bn_scale_rep", [128, C], F32, kind="ExternalInput")
    d_bnh = nc.dram_tensor("bn_shift_rep", [128, C], F32, kind="ExternalInput")
    d_rep6 = nc.dram_tensor("rep6", [6, C], F32, kind="ExternalInput")
    d_gidx = nc.dram_tensor("gidx", [128, BL * ICOLS], I16,
                            kind="ExternalInput")
    d_l1a = nc.dram_tensor("l1aug", [BL, 5, G], F32, kind="ExternalInput")
    d_l2a = nc.dram_tensor("l2aug", [BL, 5, N2], F32, kind="ExternalInput")
    d_out = nc.dram_tensor("out", [BL, G, C], F32, kind="ExternalOutput")

    from contextlib import ExitStack
    with tile.TileContext(nc) as tc:
        with tc.tile_pool(name="cpool", bufs=1) as cp, \
             tc.tile_pool(name="dram", bufs=1, space="DRAM") as dp:
            stk = ExitStack()
            wp = stk.enter_context(tc.tile_pool(name="wpool", bufs=1))
            wk = stk.enter_context(tc.tile_pool(name="work", bufs=2))
            psp = stk.enter_context(tc.tile_pool(name="ps", bufs=2,
                                                 space="PSUM"))

            # ---------------- static loads ----------------
            wqk_s = wp.tile([128, 3, 2 * C], BF16)
            nc.sync.dma_start(wqk_s[:], d_wqk.ap().rearrange(
                "(cc p) f -> p cc f", p=128))
            wv_s = wp.tile([128, 3, C], BF16)
            nc.sync.dma_start(wv_s[:], d_wv.ap().rearrange(
                "(cc p) f -> p cc f", p=128))
            pjT_s = wp.tile([128, 3, C], BF16)
            nc.sync.dma_start(pjT_s[:], d_pjT.ap().rearrange(
                "(cc p) f -> p cc f", p=128))
            w1_s = wp.tile([128, 3, HID], BF16)
            nc.sync.dma_start(w1_s[:], d_w1.ap().rearrange(
                "(cc p) f -> p cc f", p=128))
            w2_s = wp.tile([128, 12, C], BF16)
            nc.sync.dma_start(w2_s[:], d_w2.ap().rearrange(
                "(cc p) f -> p cc f", p=128))
            rep6_s = wp.tile([6, C], F32R)
            nc.sync.dma_start(rep6_s[:], d_rep6.ap().bitcast(F32R))

            qkb_s = wp.tile([128, 6], F32)
            nc.sync.dma_start(qkb_s[:], d_qkb.ap())
            f1b_s = wp.tile([128, 12], F32)
            nc.sync.dma_start(f1b_s[:], d_f1b.ap())
            vbr_s = wp.tile([128, C], F32)
            nc.sync.dma_start(vbr_s[:], d_vbr.ap())
            pbr_s = wp.tile([128, C], F32)
            nc.sync.dma_start(pbr_s[:], d_pbr.ap())
            f2br_s = wp.tile([128, C], F32)
            nc.sync.dma_start(f2br_s[:], d_f2br.ap())
            bns_s = cp.tile([128, C], F32)
            nc.sync.dma_start(bns_s[:], d_bns.ap())
            bnh_s = cp.tile([128, C], F32)
            nc.sync.dma_start(bnh_s[:], d_bnh.ap())
            gidx_s = cp.tile([128, BL, ICOLS], I16)
            nc.sync.dma_start(gidx_s[:], d_gidx.ap().rearrange(
                "p (b s) -> p b s", b=BL))
            l1a_s = cp.tile([5, BL, G], F32)
            nc.sync.dma_start(l1a_s[:], d_l1a.ap().rearrange("b r s -> r b s"))
            l2a_s = cp.tile([5, BL, N2], F32)
            nc.sync.dma_start(l2a_s[:], d_l2a.ap().rearrange("b r s -> r b s"))

            ident = cp.tile([128, 128], F32)
            make_identity(nc, ident[:])
            eps_s = cp.tile([128, 1], F32)
            nc.vector.memset(eps_s[:], EPS_LN)
            ones_s = cp.tile([128, 4], F32)
            nc.vector.memset(ones_s[:], 1.0)

            # persistent per-batch outputs
            x2k = [cp.tile([128, 4, C], BF16, name=f"x2k{b}")
                   for b in range(BL)]
            wtsb = [cp.tile([128, 4, N2], F32R, name=f"wtsb{b}")
                    for b in range(BL)]

            # internal DRAM
            ag_in = dp.tile([BL * G, C], BF16)
            table = dp.tile([ROWS, C], BF16)

            # =================== PHASE A: transformer ===================
            for b in range(BL):
                xr = wk.tile([128, 4, C], F32, name=f"xr{b}", tag="xr")
                nc.sync.dma_start(xr[:], d_x.ap()[b].rearrange(
                    "(ch p) c -> p ch c", p=128))
                # ---- LN1 -> xn (normalized; affine folded into weights) ----
                xn = wk.tile([128, 4, C], BF16, name=f"xn{b}", tag="xn",
                             bufs=2)
                for ch in range(4):
                    st6 = wk.tile([128, 6], F32, name=f"st{b}{ch}", tag="st")
                    nc.vector.bn_stats(out=st6[:], in_=xr[:, ch, :])
                    mv = wk.tile([128, 2], F32, name=f"mv{b}{ch}", tag="mv")
                    nc.vector.bn_aggr(out=mv[:], in_=st6[:])
                    sd = wk.tile([128, 1], F32, name=f"sd{b}{ch}", tag="sd")
                    nc.scalar.activation(sd[:], mv[:, 1:2], AF.Sqrt,
                                         bias=eps_s[:])
                    rs = wk.tile([128, 1], F32, name=f"rg{b}{ch}", tag="rg")
                    nc.vector.reciprocal(rs[:], sd[:])
                    nc.vector.tensor_scalar(out=xn[:, ch, :], in0=xr[:, ch, :],
                                            scalar1=mv[:, 0:1], scalar2=rs[:],
                                            op0=OP.subtract, op1=OP.mult)
                # ---- transpose xn -> xnT [c, s] ----
                xnT = wk.tile([128, 3, G], BF16, name=f"xnT{b}", tag="xnT",
                              bufs=2)
                for ch in range(4):
                    eng = nc.sync if ch % 2 == 0 else nc.scalar
                    eng.dma_start_transpose(
                        xnT[:, :, ch * 128:(ch + 1) * 128], xn[:, ch, :])

                # ---- qkT = Weff_qk @ xnT + bias ----
                qkT = wk.tile([128, 6, G], BF16, name=f"qkT{b}", tag="qkT",
                              bufs=1)
                for f in range(6):
                    ps1 = psp.tile([128, G], F32, name=f"qk{b}{f}", tag="ps_a")
                    for cc in range(3):
                        nc.tensor.matmul(ps1[:],
                                         wqk_s[:, cc, f * 128:(f + 1) * 128],
                                         xnT[:, cc, :],
                                         start=(cc == 0), stop=(cc == 2))
                    nc.vector.tensor_scalar(out=qkT[:, f, :], in0=ps1[:],
                                            scalar1=qkb_s[:, f:f + 1],
                                            scalar2=None, op0=OP.add)

                # ---- v = xn @ WvT + bias, stored as vaug [s, h, 65] ----
                vaug = wk.tile([128, 4, 6, 65], BF16, name=f"va{b}", tag="va",
                               bufs=1)
                nc.vector.memset(vaug[:, :, :, 64:65], 1.0)
                for sch in range(4):
                    ps2 = psp.tile([128, C], F32, name=f"v{b}{sch}",
                                   tag="ps_b")
                    for cc in range(3):
                        nc.tensor.matmul(ps2[:],
                                         xnT[:, cc, sch * 128:(sch + 1) * 128],
                                         wv_s[:, cc, :],
                                         start=(cc == 0), stop=(cc == 2))
                    nc.vector.tensor_tensor(
                        out=vaug[:, sch, :, 0:64],
                        in0=ps2[:].rearrange("p (h d) -> p h d", h=6),
                        in1=vbr_s[:].rearrange("p (h d) -> p h d", h=6),
                        op=OP.add)

                # ---- 3NN interpolation weights for batch b ----
                # Stage-interleaved across the 4 row-chunks so the
                # vector<->gpsimd ping-pong overlaps instead of serializing.
                # d2 = l1aug^T @ l2aug; top-3 min mask; w = mask/(d2+eps);
                # transposed un-normalized weights to wtsb (rowsum comes from
                # the ones column of vis_aug in phase C).
                d2s_t, m1_t, k1_t, da_t, m2_t = [], [], [], [], []
                k2_t, db_t, m3_t, kk_t, de_t, wi_t, w0_t = [], [], [], [], [], [], []
                for ch in range(4):
                    psd = psp.tile([128, N2], F32, name=f"d2{b}{ch}",
                                   tag="ps_c")
                    nc.tensor.matmul(psd[:],
                                     l1a_s[:, b, ch * 128:(ch + 1) * 128],
                                     l2a_s[:, b, :], start=True, stop=True)
                    d2s = wk.tile([128, N2], F32, name=f"d2s{b}{ch}",
                                  tag=f"d2s{ch}", bufs=1)
                    nc.scalar.copy(d2s[:], psd[:])
                    d2s_t.append(d2s)
                for ch in range(4):
                    m1 = wk.tile([128, 1], F32, name=f"m1{b}{ch}",
                                 tag=f"m1{ch}", bufs=1)
                    nc.vector.tensor_reduce(out=m1[:], in_=d2s_t[ch][:],
                                            axis=AX.X, op=OP.min)
                    m1_t.append(m1)
                for ch in range(4):
                    msk1 = wk.tile([128, N2], F32, name=f"k1{b}{ch}",
                                   tag=f"k1{ch}", bufs=1)
                    nc.vector.tensor_scalar(out=msk1[:], in0=d2s_t[ch][:],
                                            scalar1=m1_t[ch][:], scalar2=BIG,
                                            op0=OP.is_le, op1=OP.mult)
                    k1_t.append(msk1)
                for ch in range(4):
                    d2a = wk.tile([128, N2], F32, name=f"da{b}{ch}",
                                  tag=f"da{ch}", bufs=1)
                    nc.vector.tensor_tensor(out=d2a[:], in0=d2s_t[ch][:],
                                            in1=k1_t[ch][:], op=OP.add)
                    da_t.append(d2a)
                for ch in range(4):
                    m2 = wk.tile([128, 1], F32, name=f"m2{b}{ch}",
                                 tag=f"m2{ch}", bufs=1)
                    nc.vector.tensor_reduce(out=m2[:], in_=da_t[ch][:],
                                            axis=AX.X, op=OP.min)
                    m2_t.append(m2)
                for ch in range(4):
                    msk2 = wk.tile([128, N2], F32, name=f"k2{b}{ch}",
                                   tag=f"k1{ch}", bufs=1)
                    nc.vector.tensor_scalar(out=msk2[:], in0=da_t[ch][:],
                                            scalar1=m2_t[ch][:], scalar2=BIG,
                                            op0=OP.is_le, op1=OP.mult)
                    k2_t.append(msk2)
                for ch in range(4):
                    nc.vector.tensor_tensor(out=da_t[ch][:], in0=da_t[ch][:],
                                            in1=k2_t[ch][:], op=OP.add)
                    db_t.append(da_t[ch])
                for ch in range(4):
                    m3 = wk.tile([128, 1], F32, name=f"m3{b}{ch}",
                                 tag=f"m3{ch}", bufs=1)
                    nc.vector.tensor_reduce(out=m3[:], in_=db_t[ch][:],
                                            axis=AX.X, op=OP.min)
                    m3_t.append(m3)
                for ch in range(4):
                    msk = wk.tile([128, N2], F32, name=f"kk{b}{ch}",
                                  tag=f"k1{ch}", bufs=1)
                    nc.vector.tensor_scalar(out=msk[:], in0=d2s_t[ch][:],
                                            scalar1=m3_t[ch][:], scalar2=None,
                                            op0=OP.is_le)
                    kk_t.append(msk)
                    d2e = wk.tile([128, N2], F32, name=f"de{b}{ch}",
                                  tag=f"da{ch}", bufs=1)
                    nc.vector.tensor_scalar(out=d2e[:], in0=d2s_t[ch][:],
                                            scalar1=EPS_W, scalar2=None,
                                            op0=OP.add)
                    de_t.append(d2e)
                for ch in range(4):
                    wiv = wk.tile([128, N2], F32, name=f"wi{b}{ch}",
                                  tag=f"wi{ch}", bufs=1)
                    nc.vector.reciprocal(wiv[:], de_t[ch][:])
                    wi_t.append(wiv)
                for ch in range(4):
                    w0 = wk.tile([128, N2], F32, name=f"w0{b}{ch}",
                                 tag=f"w0{ch}", bufs=1)
                    nc.vector.tensor_tensor(out=w0[:], in0=kk_t[ch][:],
                                            in1=wi_t[ch][:], op=OP.mult)
                    w0_t.append(w0)
                for ch in range(4):
                    pst = psp.tile([128, N2], F32, name=f"wt{b}{ch}",
                                   tag="ps_c")
                    nc.tensor.transpose(pst[:], w0_t[ch][:], ident[:])
                    nc.vector.tensor_copy(wtsb[b][:, ch, :], pst[:])

                # ---- attention per head ----
                oTr = wk.tile([128, 3, G], BF16, name=f"oTr{b}", tag="oTr",
                              bufs=2)
                sums = wk.tile([1, 6, G], F32, name=f"sm{b}", tag="sm",
                               bufs=1)
                for h in range(6):
                    po = (h % 2) * 64
                    qT = qkT[po:po + 64, h // 2, :]
                    kT = qkT[po:po + 64, 3 + h // 2, :]
                    Eh = wk.tile([128, 4, G], BF16, name=f"E{b}{h}", tag="E",
                                 bufs=2)
                    for kc in range(4):
                        ps3 = psp.tile([128, G], F32, name=f"s{b}{h}{kc}",
                                       tag="ps_a")
                        nc.tensor.matmul(ps3[:],
                                         kT[:, kc * 128:(kc + 1) * 128],
                                         qT, start=True, stop=True)
                        nc.scalar.activation(Eh[:, kc, :], ps3[:], AF.Exp)
                    ps4 = psp.tile([65, G], F32, name=f"o{b}{h}", tag="ps_c")
                    for kc in range(4):
                        nc.tensor.matmul(ps4[:], vaug[:, kc, h, :],
                                         Eh[:, kc, :],
                                         start=(kc == 0), stop=(kc == 3))
                    nc.scalar.copy(oTr[po:po + 64, h // 2, :], ps4[0:64, :])
                    nc.scalar.copy(sums[0:1, h, :], ps4[64:65, :])

                # ---- normalization matrix R, scale oT ----
                sums6 = wk.tile([6, G], F32, name=f"s6{b}", tag="s6", bufs=1)
                nc.sync.dma_start(sums6[:], sums[0:1, :, :])
                srec = wk.tile([6, G], F32R, name=f"sr{b}", tag="sr", bufs=1)
                with nc.allow_low_precision("fp32r is fp32-width"):
                    nc.vector.reciprocal(srec[:], sums6[:])
                oTs = wk.tile([128, 3, G], BF16, name=f"oTs{b}", tag="oTs",
                              bufs=1)
                for cc in range(3):
                    ps5 = psp.tile([128, G], F32, name=f"R{b}{cc}", tag="ps_a")
                    nc.tensor.matmul(ps5[:],
                                     rep6_s[:, cc * 128:(cc + 1) * 128],
                                     srec[:], start=True, stop=True)
                    nc.vector.tensor_tensor(out=oTs[:, cc, :],
                                            in0=oTr[:, cc, :], in1=ps5[:],
                                            op=OP.mult)

                # ---- proj + residual -> x1 ----
                x1 = wk.tile([128, 4, C], F32, name=f"x1{b}", tag="x1",
                             bufs=2)
                for sch in range(4):
                    ps6 = psp.tile([128, C], F32, name=f"pj{b}{sch}",
                                   tag="ps_b")
                    for cc in range(3):
                        nc.tensor.matmul(ps6[:],
                                         oTs[:, cc, sch * 128:(sch + 1) * 128],
                                         pjT_s[:, cc, :],
                                         start=(cc == 0), stop=(cc == 2))
                    nc.vector.tensor_tensor(out=x1[:, sch, :], in0=ps6[:],
                                            in1=xr[:, sch, :], op=OP.add)

                pb_b = bass.AP(pbr_s.tensor, pbr_s[:].offset,
                               [pbr_s[:].ap[0], [0, 4], pbr_s[:].ap[1]])
                nc.vector.tensor_tensor(out=x1[:], in0=x1[:], in1=pb_b,
                                        op=OP.add)

                # ---- LN2 -> xn2 ----
                xn2 = wk.tile([128, 4, C], BF16, name=f"xn2{b}", tag="xn",
                              bufs=2)
                for ch in range(4):
                    st6b = wk.tile([128, 6], F32, name=f"su{b}{ch}", tag="st")
                    nc.vector.bn_stats(out=st6b[:], in_=x1[:, ch, :])
                    mvb = wk.tile([128, 2], F32, name=f"mw{b}{ch}", tag="mv")
                    nc.vector.bn_aggr(out=mvb[:], in_=st6b[:])
                    sdb = wk.tile([128, 1], F32, name=f"se{b}{ch}", tag="sd")
                    nc.scalar.activation(sdb[:], mvb[:, 1:2], AF.Sqrt,
                                         bias=eps_s[:])
                    rsb = wk.tile([128, 1], F32, name=f"rh{b}{ch}", tag="rg")
                    nc.vector.reciprocal(rsb[:], sdb[:])
                    nc.vector.tensor_scalar(out=xn2[:, ch, :],
                                            in0=x1[:, ch, :],
                                            scalar1=mvb[:, 0:1],
                                            scalar2=rsb[:],
                                            op0=OP.subtract, op1=OP.mult)
                xn2T = wk.tile([128, 3, G], BF16, name=f"x2T{b}", tag="xnT",
                               bufs=2)
                for ch in range(4):
                    eng = nc.sync if ch % 2 == 0 else nc.scalar
                    eng.dma_start_transpose(
                        xn2T[:, :, ch * 128:(ch + 1) * 128], xn2[:, ch, :])

                # ---- fc1 + gelu -> uT ----
                uT = wk.tile([128, 12, G], BF16, name=f"uT{b}", tag="uT",
                             bufs=1)
                for f in range(12):
                    ps7 = psp.tile([128, G], F32, name=f"f1{b}{f}", tag="ps_a")
                    for cc in range(3):
                        nc.tensor.matmul(ps7[:],
                                         w1_s[:, cc, f * 128:(f + 1) * 128],
                                         xn2T[:, cc, :],
                                         start=(cc == 0), stop=(cc == 2))
                    nc.scalar.activation(uT[:, f, :], ps7[:], AF.Gelu,
                                         bias=f1b_s[:, f:f + 1])

                # ---- fc2 + residual -> x2 (bf16, kept in SBUF) ----
                for sch in range(4):
                    ps8 = psp.tile([128, C], F32, name=f"f2{b}{sch}",
                                   tag="ps_b")
                    for f in range(12):
                        nc.tensor.matmul(ps8[:],
                                         uT[:, f, sch * 128:(sch + 1) * 128],
                                         w2_s[:, f, :],
                                         start=(f == 0), stop=(f == 11))
                    x2c = wk.tile([128, C], F32, name=f"x2{b}{sch}",
                                  tag="x2c", bufs=2)
                    nc.vector.tensor_tensor(out=x2c[:], in0=ps8[:],
                                            in1=x1[:, sch, :], op=OP.add)
                    nc.vector.tensor_tensor(out=x2k[b][:, sch, :], in0=x2c[:],
                                            in1=f2br_s[:], op=OP.add)
                    row0 = b * G + sch * 128
                    nc.sync.dma_start(ag_in[row0:row0 + 128, :],
                                      x2k[b][:, sch, :])
                if b % CHB == CHB - 1:
                    j = b // CHB
                    nc.gpsimd.collective_compute(
                        "AllGather", OP.bypass,
                        replica_groups=[list(range(NCORES))],
                        ins=[ag_in[(b - CHB + 1) * G:(b + 1) * G, :]],
                        outs=[table[j * CHROWS:(j + 1) * CHROWS, :]])

            # =================== PHASE C: gather/pool/3NN ===============
            stk.close()
            stk2 = ExitStack()
            gp = stk2.enter_context(tc.tile_pool(name="gat", bufs=2))
            ps2p = stk2.enter_context(
                tc.tile_pool(name="psC", bufs=4, space="PSUM"))
            for b in range(BL):
                acc = gp.tile([128, K + 1, C], BF16, name=f"acc{b}",
                              tag="acc", bufs=2)
                for g in range(5):
                    s0, s1 = g * 8, min(K + 1, (g + 1) * 8)
                    n = (s1 - s0) * 128
                    nc.gpsimd.dma_gather(
                        acc[:, s0:s1, :], table[:, :],
                        gidx_s[:, b, s0 * 8:s1 * 8], n, n, C)
                # max-pool tree over K=32 neighbor slots (in-place halving)
                for half in (16, 8, 4, 2, 1):
                    nc.vector.tensor_tensor(
                        out=acc[:, 0:half, :], in0=acc[:, 0:half, :],
                        in1=acc[:, half:2 * half, :], op=OP.max)
                # BN (x2 & affine folded) + gelu + 0.3*centers, ones col
                pb1 = gp.tile([128, C], F32, name=f"pb1{b}", tag="pb1")
                nc.vector.tensor_tensor(out=pb1[:], in0=acc[:, 0, :],
                                        in1=bns_s[:], op=OP.mult)
                pb2 = gp.tile([128, C], F32, name=f"pb2{b}", tag="pb2")
                nc.vector.tensor_tensor(out=pb2[:], in0=pb1[:], in1=bnh_s[:],
                                        op=OP.add)
                gl = gp.tile([128, C], F32, name=f"gl{b}", tag="gl")
                nc.scalar.activation(gl[:], pb2[:], AF.Gelu)
                visa = gp.tile([128, C + 4], F32R, name=f"vis{b}", tag="vis")
                nc.vector.tensor_copy(visa[:, C:C + 4], ones_s[:])
                nc.vector.scalar_tensor_tensor(
                    out=visa[:, 0:C], in0=acc[:, K, :], scalar=0.3, in1=gl[:],
                    op0=OP.mult, op1=OP.add)

                for ch in range(4):
                    psi = ps2p.tile([128, C + 4], F32, name=f"ip{b}{ch}",
                                    tag="ps_i")
                    nc.tensor.matmul(psi[:], wtsb[b][:, ch, :], visa[:],
                                     start=True, stop=True)
                    rsm = gp.tile([128, 1], F32, name=f"rm{b}{ch}", tag="rm")
                    nc.vector.reciprocal(rsm[:], psi[:, C:C + 1])
                    ocs = gp.tile([128, C], F32, name=f"os{b}{ch}", tag="os")
                    nc.vector.tensor_scalar(out=ocs[:], in0=psi[:, 0:C],
                                            scalar1=rsm[:], scalar2=None,
                                            op0=OP.mult)
                    oc = gp.tile([128, C], F32, name=f"oc{b}{ch}", tag="oc")
                    nc.vector.tensor_tensor(out=oc[:], in0=ocs[:],
                                            in1=x2k[b][:, ch, :], op=OP.add)
                    nc.sync.dma_start(
                        d_out.ap()[b, ch * 128:(ch + 1) * 128, :], oc[:])
            stk2.close()

    nc.compile()
    return nc


def _prep_inputs(x, level1_center, level2_center, ln1_g, ln1_b, qkv_w, proj_w,
                 proj_b, ln2_g, ln2_b, fc1_w, fc1_b, fc2_w, fc2_b, bn_g, bn_b,
                 bn_mean, bn_var, level1_index, level2_index):
    """Build the per-core in_maps (host-side folding + sharding)."""
    f32 = np.float32
    x = np.ascontiguousarray(np.asarray(x, f32))
    l1c = np.asarray(level1_center, f32)
    l2c = np.asarray(level2_center, f32)
    ln1_g = np.asarray(ln1_g, f32); ln1_b = np.asarray(ln1_b, f32)
    ln2_g = np.asarray(ln2_g, f32); ln2_b = np.asarray(ln2_b, f32)
    qkv_w = np.asarray(qkv_w, f32); proj_w = np.asarray(proj_w, f32)
    proj_b = np.asarray(proj_b, f32)
    fc1_w = np.asarray(fc1_w, f32); fc1_b = np.asarray(fc1_b, f32)
    fc2_w = np.asarray(fc2_w, f32); fc2_b = np.asarray(fc2_b, f32)
    bn_g = np.asarray(bn_g, f32); bn_b = np.asarray(bn_b, f32)
    bn_mean = np.asarray(bn_mean, f32); bn_var = np.asarray(bn_var, f32)
    l1i = np.asarray(level1_index).astype(np.int64).reshape(B, N2, K)
    l2i = np.asarray(level2_index).astype(np.int64).reshape(B, N2)

    # remap global row ids to the 4-chunk AllGather table layout:
    # chunk j holds local batches {2j, 2j+1} of every core.
    def _remap(r):
        c = r // (BL * G)
        rem = r % (BL * G)
        b = rem // G
        g = rem % G
        return ((b // CHB) * CHROWS + c * (CHB * G) + (b % CHB) * G + g)

    l1i = _remap(l1i)
    l2i = _remap(l2i)

    s = HD ** -0.5
    weff = qkv_w * ln1_g[None, :]
    beff = qkv_w @ ln1_b
    weff[:C] *= s
    beff[:C] *= s
    wqkT = np.ascontiguousarray(weff[:2 * C].T.astype(ml_dtypes.bfloat16))
    qk_bias = np.ascontiguousarray(beff[:2 * C].reshape(6, 128).T)
    wvT = np.ascontiguousarray(weff[2 * C:].T.astype(ml_dtypes.bfloat16))
    v_bias_rep = np.ascontiguousarray(
        np.broadcast_to(beff[2 * C:], (128, C)))
    projT = np.ascontiguousarray(proj_w.T.astype(ml_dtypes.bfloat16))
    proj_b_rep = np.ascontiguousarray(np.broadcast_to(proj_b, (128, C)))
    w1eff = fc1_w * ln2_g[None, :]
    f1bias = fc1_b + fc1_w @ ln2_b
    wfc1T = np.ascontiguousarray(w1eff.T.astype(ml_dtypes.bfloat16))
    fc1_bias = np.ascontiguousarray(f1bias.reshape(12, 128).T)
    wfc2T = np.ascontiguousarray(fc2_w.T.astype(ml_dtypes.bfloat16))
    fc2_b_rep = np.ascontiguousarray(np.broadcast_to(fc2_b, (128, C)))
    gs = bn_g / np.sqrt(bn_var + EPS_BN)
    bn_scale_rep = np.ascontiguousarray(
        np.broadcast_to((2.0 * gs).astype(f32), (128, C)))
    bn_shift_rep = np.ascontiguousarray(
        np.broadcast_to((bn_b - bn_mean * gs).astype(f32), (128, C)))
    rep6 = np.zeros((6, C), f32)
    for h in range(H):
        rep6[h, h * HD:(h + 1) * HD] = 1.0

    # 3NN augmented coordinate blocks
    l1n = (l1c ** 2).sum(-1)                                 # [B, G]
    l2n = (l2c ** 2).sum(-1)                                 # [B, N2]
    l1aug = np.empty((B, 5, G), f32)
    l1aug[:, 0:3] = np.transpose(l1c, (0, 2, 1))
    l1aug[:, 3] = 1.0
    l1aug[:, 4] = l1n
    l2aug = np.empty((B, 5, N2), f32)
    l2aug[:, 0:3] = -2.0 * np.transpose(l2c, (0, 2, 1))
    l2aug[:, 3] = l2n
    l2aug[:, 4] = 1.0

    shared = {
        "wqkT": wqkT, "qk_bias": qk_bias, "wvT": wvT,
        "v_bias_rep": v_bias_rep, "projT": projT,
        "proj_b_rep": proj_b_rep, "wfc1T": wfc1T, "fc1_bias": fc1_bias,
        "wfc2T": wfc2T, "fc2_b_rep": fc2_b_rep,
        "bn_scale_rep": bn_scale_rep, "bn_shift_rep": bn_shift_rep,
        "rep6": rep6,
    }
    in_maps = []
    for c in range(NCORES):
        b0 = c * BL
        # bulk-gather index tile: [128, BL*ICOLS] int16, wrapped in 16
        # partitions, replicated 8x across partition groups (one per Q7 core).
        gidx = np.empty((128, BL * ICOLS), np.int16)
        for b in range(BL):
            idxs = np.empty((NI,), np.int64)
            # slot j<K at i=j*128+p -> neighbor j of point p; j=K -> center
            idxs[:K * 128] = np.transpose(
                l1i[b0 + b], (1, 0)).reshape(K * 128)
            idxs[K * 128:] = l2i[b0 + b]
            wrap = idxs.reshape(ICOLS, 16).T.astype(np.int16)
            gidx[:, b * ICOLS:(b + 1) * ICOLS] = np.tile(wrap, (8, 1))
        m = dict(shared)
        m["x_in"] = np.ascontiguousarray(x[b0:b0 + BL])
        m["gidx"] = gidx
        m["l1aug"] = np.ascontiguousarray(l1aug[b0:b0 + BL])
        m["l2aug"] = np.ascontiguousarray(l2aug[b0:b0 + BL])
        in_maps.append(m)
    return in_maps


def get_program():
    if "nc" not in _CACHE:
        _CACHE["nc"] = _build_program()
    return _CACHE["nc"]


def run(in_maps, **kw):
    nc = get_program()
    return bass_utils.run_bass_kernel_spmd(
        nc, in_maps, core_ids=list(range(NCORES)), **kw)


def kernel(**inputs):
    in_maps = _prep_inputs(**inputs)
    res = run(in_maps)
    out = np.concatenate([res.results[c]["out"] for c in range(NCORES)],
                         axis=0)
    return out.astype(np.float32)


if __name__ == "__main__":
    np.random.seed(0)
    get_program()
    print("program built + compiled OK")


# revision 3
# speedup vs baseline: 1.1445x; 1.1347x over previous
"""Trainium2 Bass kernel for nn_Block_86921548136965 (gnn_message_passing), v2.

Transformer block (LN->MHA->LN->MLP) + global neighbor max-pool + BN/GELU +
3-NN inverse-distance interpolation, data-parallel over batch across 8
NeuronCores.

v2 changes vs baseline:
- Phase C neighbor gather via one bulk dma_gather per batch (vs 33 SWDGE
  indirect DMAs each) - removes the ~360us serial gpsimd bottleneck.
- 4-chunk AllGather (after local batches 1,3,5,7) so only the last ~3MB
  chunk is exposed.
- 3NN interpolation weights precomputed during Phase A on the otherwise
  idle gpsimd/vector engines.
- x2 kept in SBUF as bf16 (no fp32 DRAM round trip).
- softmax-sum reshape via a single SBUF->SBUF DMA (no DRAM bounce).
- rowsum of interp weights folded into the interp matmul (ones column).
"""
import os
import sys

sys.path.insert(0, "/opt/trn_rl_repo")

import numpy as np
import ml_dtypes

import concourse.bass as bass
import concourse.bacc as bacc
import concourse.tile as tile
from concourse import mybir
from concourse import bass_utils
from concourse.masks import make_identity

# problem shapes
B, G, C, H = 64, 512, 384, 6
HD = C // H  # 64
N2, K = 128, 32
HID = 4 * C  # 1536
NCORES = 8
BL = B // NCORES  # 8 batches per core
ROWS = B * G  # 32768 global rows
CHB = 2  # batches per AllGather chunk
NCHUNK = BL // CHB  # 4
CHROWS = ROWS // NCHUNK  # 8192 rows per chunk
NI = (K + 1) * 128  # 4224 gathered rows per batch (32 neigh + 1 center)
ICOLS = NI // 16  # 264 int16 per partition (16-wrap)

F32 = mybir.dt.float32
F32R = mybir.dt.float32r
BF16 = mybir.dt.bfloat16
I16 = mybir.dt.int16
AX = mybir.AxisListType
OP = mybir.AluOpType
AF = mybir.ActivationFunctionType

BIG = 1.0e30
EPS_LN = 1e-5
EPS_BN = 1e-5
EPS_W = 1e-8

_CACHE = {}


def _build_program():
    nc = bacc.Bacc("TRN2", target_bir_lowering=False, debug=False,
                   num_devices=NCORES)

    # ---------------- DRAM I/O ----------------
    d_x = nc.dram_tensor("x_in", [BL, G, C], F32, kind="ExternalInput")
    d_wqk = nc.dram_tensor("wqkT", [C, 2 * C], BF16, kind="ExternalInput")
    d_qkb = nc.dram_tensor("qk_bias", [128, 6], F32, kind="ExternalInput")
    d_wv = nc.dram_tensor("wvT", [C, C], BF16, kind="ExternalInput")
    d_vbr = nc.dram_tensor("v_bias_rep", [128, C], F32, kind="ExternalInput")
    d_pjT = nc.dram_tensor("projT", [C, C], BF16, kind="ExternalInput")
    d_pbr = nc.dram_tensor("proj_b_rep", [128, C], F32, kind="ExternalInput")
    d_w1 = nc.dram_tensor("wfc1T", [C, HID], BF16, kind="ExternalInput")
    d_f1b = nc.dram_tensor("fc1_bias", [128, 12], F32, kind="ExternalInput")
    d_w2 = nc.dram_tensor("wfc2T", [HID, C], BF16, kind="ExternalInput")
    d_f2br = nc.dram_tensor("fc2_b_rep", [128, C], F32, kind="ExternalInput")
    d_bns = nc.dram_tensor("bn_scale_rep", [128, C], F32, kind="ExternalInput")
    d_bnh = nc.dram_tensor("bn_shift_rep", [128, C], F32, kind="ExternalInput")
    d_rep6 = nc.dram_tensor("rep6", [6, C], F32, kind="ExternalInput")
    d_gidx = nc.dram_tensor("gidx", [128, BL * ICOLS], I16,
                            kind="ExternalInput")
    d_l1a = nc.dram_tensor("l1aug", [BL, 5, G], F32, kind="ExternalInput")
    d_l2a = nc.dram_tensor("l2aug", [BL, 5, N2], F32, kind="ExternalInput")
    d_out = nc.dram_tensor("out", [BL, G, C], F32, kind="ExternalOutput")

    from contextlib import ExitStack
    with tile.TileContext(nc) as tc:
        with tc.tile_pool(name="cpool", bufs=1) as cp, \
             tc.tile_pool(name="dram", bufs=1, space="DRAM") as dp:
            stk = ExitStack()
            wp = stk.enter_context(tc.tile_pool(name="wpool", bufs=1))
            wk = stk.enter_context(tc.tile_pool(name="work", bufs=2))
            psp = stk.enter_context(tc.tile_pool(name="ps", bufs=2,
                                                 space="PSUM"))

            # ---------------- static loads ----------------
            wqk_s = wp.tile([128, 3, 2 * C], BF16)
            nc.sync.dma_start(wqk_s[:], d_wqk.ap().rearrange(
                "(cc p) f -> p cc f", p=128))
            wv_s = wp.tile([128, 3, C], BF16)
            nc.sync.dma_start(wv_s[:], d_wv.ap().rearrange(
                "(cc p) f -> p cc f", p=128))
            pjT_s = wp.tile([128, 3, C], BF16)
            nc.sync.dma_start(pjT_s[:], d_pjT.ap().rearrange(
                "(cc p) f -> p cc f", p=128))
            w1_s = wp.tile([128, 3, HID], BF16)
            nc.sync.dma_start(w1_s[:], d_w1.ap().rearrange(
                "(cc p) f -> p cc f", p=128))
            w2_s = wp.tile([128, 12, C], BF16)
            nc.sync.dma_start(w2_s[:], d_w2.ap().rearrange(
                "(cc p) f -> p cc f", p=128))
            rep6_s = wp.tile([6, C], F32R)
            nc.sync.dma_start(rep6_s[:], d_rep6.ap().bitcast(F32R))

            qkb_s = wp.tile([128, 6], F32)
            nc.sync.dma_start(qkb_s[:], d_qkb.ap())
            f1b_s = wp.tile([128, 12], F32)
            nc.sync.dma_start(f1b_s[:], d_f1b.ap())
            vbr_s = wp.tile([128, C], F32)
            nc.sync.dma_start(vbr_s[:], d_vbr.ap())
            pbr_s = wp.tile([128, C], F32)
            nc.sync.dma_start(pbr_s[:], d_pbr.ap())
            f2br_s = wp.tile([128, C], F32)
            nc.sync.dma_start(f2br_s[:], d_f2br.ap())
            bns_s = cp.tile([128, C], F32)
            nc.sync.dma_start(bns_s[:], d_bns.ap())
            bnh_s = cp.tile([128, C], F32)
            nc.sync.dma_start(bnh_s[:], d_bnh.ap())
            gidx_s = cp.tile([128, BL, ICOLS], I16)
            nc.sync.dma_start(gidx_s[:], d_gidx.ap().rearrange(
                "p (b s) -> p b s", b=BL))
            l1a_s = cp.tile([5, BL, G], F32)
            nc.sync.dma_start(l1a_s[:], d_l1a.ap().rearrange("b r s -> r b s"))
            l2a_s = cp.tile([5, BL, N2], F32)
            nc.sync.dma_start(l2a_s[:], d_l2a.ap().rearrange("b r s -> r b s"))

            ident = cp.tile([128, 128], F32)
            make_identity(nc, ident[:])
            eps_s = cp.tile([128, 1], F32)
            nc.vector.memset(eps_s[:], EPS_LN)
            ones_s = cp.tile([128, 4], F32)
            nc.vector.memset(ones_s[:], 1.0)

            # persistent per-batch outputs
            x2k = [cp.tile([128, 4, C], BF16, name=f"x2k{b}")
                   for b in range(BL)]
            wtsb = [cp.tile([128, 4, N2], F32R, name=f"wtsb{b}")
                    for b in range(BL)]

            # internal DRAM
            ag_in = dp.tile([BL * G, C], BF16)
            table = dp.tile([ROWS, C], BF16)

            # =================== PHASE A: transformer ===================
            for b in range(BL):
                xr = wk.tile([128, 4, C], F32, name=f"xr{b}", tag="xr")
                nc.sync.dma_start(xr[:], d_x.ap()[b].rearrange(
                    "(ch p) c -> p ch c", p=128))
                # ---- LN1 -> xn (normalized; affine folded into weights) ----
                xn = wk.tile([128, 4, C], BF16, name=f"xn{b}", tag="xn",
                             bufs=2)
                for ch in range(4):
                    st6 = wk.tile([128, 6], F32, name=f"st{b}{ch}", tag="st")
                    nc.vector.bn_stats(out=st6[:], in_=xr[:, ch, :])
                    mv = wk.tile([128, 2], F32, name=f"mv{b}{ch}", tag="mv")
                    nc.vector.bn_aggr(out=mv[:], in_=st6[:])
                    sd = wk.tile([128, 1], F32, name=f"sd{b}{ch}", tag="sd")
                    nc.scalar.activation(sd[:], mv[:, 1:2], AF.Sqrt,
                                         bias=eps_s[:])
                    rs = wk.tile([128, 1], F32, name=f"rg{b}{ch}", tag="rg")
                    nc.vector.reciprocal(rs[:], sd[:])
                    nc.vector.tensor_scalar(out=xn[:, ch, :], in0=xr[:, ch, :],
                                            scalar1=mv[:, 0:1], scalar2=rs[:],
                                            op0=OP.subtract, op1=OP.mult)
                # ---- transpose xn -> xnT [c, s] ----
                xnT = wk.tile([128, 3, G], BF16, name=f"xnT{b}", tag="xnT",
                              bufs=2)
                for ch in range(4):
                    eng = nc.sync if ch % 2 == 0 else nc.scalar
                    eng.dma_start_transpose(
                        xnT[:, :, ch * 128:(ch + 1) * 128], xn[:, ch, :])

                # ---- qkT = Weff_qk @ xnT + bias ----
                qkT = wk.tile([128, 6, G], BF16, name=f"qkT{b}", tag="qkT",
                              bufs=1)
                for f in range(6):
                    ps1 = psp.tile([128, G], F32, name=f"qk{b}{f}", tag="ps_a")
                    for cc in range(3):
                        nc.tensor.matmul(ps1[:],
                                         wqk_s[:, cc, f * 128:(f + 1) * 128],
                                         xnT[:, cc, :],
                                         start=(cc == 0), stop=(cc == 2))
                    nc.vector.tensor_scalar(out=qkT[:, f, :], in0=ps1[:],
                                            scalar1=qkb_s[:, f:f + 1],
                                            scalar2=None, op0=OP.add)

                # ---- v = xn @ WvT + bias, stored as vaug [s, h, 65] ----
                vaug = wk.tile([128, 4, 6, 65], BF16, name=f"va{b}", tag="va",
                               bufs=1)
                nc.vector.memset(vaug[:, :, :, 64:65], 1.0)
                for sch in range(4):
                    ps2 = psp.tile([128, C], F32, name=f"v{b}{sch}",
                                   tag="ps_b")
                    for cc in range(3):
                        nc.tensor.matmul(ps2[:],
                                         xnT[:, cc, sch * 128:(sch + 1) * 128],
                                         wv_s[:, cc, :],
                                         start=(cc == 0), stop=(cc == 2))
                    nc.vector.tensor_tensor(
                        out=vaug[:, sch, :, 0:64],
                        in0=ps2[:].rearrange("p (h d) -> p h d", h=6),
                        in1=vbr_s[:].rearrange("p (h d) -> p h d", h=6),
                        op=OP.add)

                # ---- 3NN interpolation weights for batch b ----
                # Stage-interleaved across the 4 row-chunks so the
                # vector<->gpsimd ping-pong overlaps instead of serializing.
                # d2 = l1aug^T @ l2aug; top-3 min mask; w = mask/(d2+eps);
                # transposed un-normalized weights to wtsb (rowsum comes from
                # the ones column of vis_aug in phase C).
                d2s_t, m1_t, k1_t, da_t, m2_t = [], [], [], [], []
                k2_t, db_t, m3_t, kk_t, de_t, wi_t, w0_t = [], [], [], [], [], [], []
                for ch in range(4):
                    psd = psp.tile([128, N2], F32, name=f"d2{b}{ch}",
                                   tag="ps_c")
                    nc.tensor.matmul(psd[:],
                                     l1a_s[:, b, ch * 128:(ch + 1) * 128],
                                     l2a_s[:, b, :], start=True, stop=True)
                    d2s = wk.tile([128, N2], F32, name=f"d2s{b}{ch}",
                                  tag=f"d2s{ch}", bufs=1)
                    nc.scalar.copy(d2s[:], psd[:])
                    d2s_t.append(d2s)
                for ch in range(4):
                    m1 = wk.tile([128, 1], F32, name=f"m1{b}{ch}",
                                 tag=f"m1{ch}", bufs=1)
                    nc.vector.tensor_reduce(out=m1[:], in_=d2s_t[ch][:],
                                            axis=AX.X, op=OP.min)
                    m1_t.append(m1)
                for ch in range(4):
                    msk1 = wk.tile([128, N2], F32, name=f"k1{b}{ch}",
                                   tag=f"k1{ch}", bufs=1)
                    nc.vector.tensor_scalar(out=msk1[:], in0=d2s_t[ch][:],
                                            scalar1=m1_t[ch][:], scalar2=BIG,
                                            op0=OP.is_le, op1=OP.mult)
                    k1_t.append(msk1)
                for ch in range(4):
                    d2a = wk.tile([128, N2], F32, name=f"da{b}{ch}",
                                  tag=f"da{ch}", bufs=1)
                    nc.vector.tensor_tensor(out=d2a[:], in0=d2s_t[ch][:],
                                            in1=k1_t[ch][:], op=OP.add)
                    da_t.append(d2a)
                for ch in range(4):
                    m2 = wk.tile([128, 1], F32, name=f"m2{b}{ch}",
                                 tag=f"m2{ch}", bufs=1)
                    nc.vector.tensor_reduce(out=m2[:], in_=da_t[ch][:],
                                            axis=AX.X, op=OP.min)
                    m2_t.append(m2)
                for ch in range(4):
                    msk2 = wk.tile([128, N2], F32, name=f"k2{b}{ch}",
                                   tag=f"k1{ch}", bufs=1)
                    nc.vector.tensor_scalar(out=msk2[:], in0=da_t[ch][:],
                                            scalar1=m2_t[ch][:], scalar2=BIG,
                                            op0=OP.is_le, op1=OP.mult)
                    k2_t.append(msk2)
                for ch in range(4):
                    nc.vector.tensor_tensor(out=da_t[ch][:], in0=da_t[ch][:],
                                            in1=k2_t[ch][:], op=OP.add)
                    db_t.append(da_t[ch])
                for ch in range(4):
                    m3 = wk.tile([128, 1], F32, name=f"m3{b}{ch}",
                                 tag=f"m3{ch}", bufs=1)
                    nc.vector.tensor_reduce(out=m3[:], in_=db_t[ch][:],
                                            axis=AX.X, op=OP.min)
                    m3_t.append(m3)
                for ch in range(4):
                    msk = wk.tile([128, N2], F32, name=f"kk{b}{ch}",
                                  tag=f"k1{ch}", bufs=1)
                    nc.vector.tensor_scalar(out=msk[:], in0=d2s_t[ch][:],
                                            scalar1=m3_t[ch][:], scalar2=None,
                                            op0=OP.is_le)
                    kk_t.append(msk)
                    d2e = wk.tile([128, N2], F32, name=f"de{b}{ch}",
                                  tag=f"da{ch}", bufs=1)
                    nc.vector.tensor_scalar(out=d2e[:], in0=d2s_t[ch][:],
                                            scalar1=EPS_W, scalar2=None,
                                            op0=OP.add)
                    de_t.append(d2e)
                for ch in range(4):
                    wiv = wk.tile([128, N2], F32, name=f"wi{b}{ch}",
                                  tag=f"wi{ch}", bufs=1)
                    nc.vector.reciprocal(wiv[:], de_t[ch][:])
                    wi_t.append(wiv)
                for ch in range(4):
                    w0 = wk.tile([128, N2], F32, name=f"w0{b}{ch}",
                                 tag=f"w0{ch}", bufs=1)
                    nc.vector.tensor_tensor(out=w0[:], in0=kk_t[ch][:],
                                            in1=wi_t[ch][:], op=OP.mult)
                    w0_t.append(w0)
                for ch in range(4):
                    pst = psp.tile([128, N2], F32, name=f"wt{b}{ch}",
                                   tag="ps_c")
                    nc.tensor.transpose(pst[:], w0_t[ch][:], ident[:])
                    nc.vector.tensor_copy(wtsb[b][:, ch, :], pst[:])

                # ---- attention per head ----
                oTr = wk.tile([128, 3, G], BF16, name=f"oTr{b}", tag="oTr",
                              bufs=2)
                sums = wk.tile([1, 6, G], F32, name=f"sm{b}", tag="sm",
                               bufs=1)
                for h in range(6):
                    po = (h % 2) * 64
                    qT = qkT[po:po + 64, h // 2, :]
                    kT = qkT[po:po + 64, 3 + h // 2, :]
                    Eh = wk.tile([128, 4, G], BF16, name=f"E{b}{h}", tag="E",
                                 bufs=3)
                    for kc in range(4):
                        ps3 = psp.tile([128, G], F32, name=f"s{b}{h}{kc}",
                                       tag="ps_a")
                        nc.tensor.matmul(ps3[:],
                                         kT[:, kc * 128:(kc + 1) * 128],
                                         qT, start=True, stop=True)
                        nc.scalar.activation(Eh[:, kc, :], ps3[:], AF.Exp)
                    ps4 = psp.tile([65, G], F32, name=f"o{b}{h}", tag="ps_c")
                    for kc in range(4):
                        nc.tensor.matmul(ps4[:], vaug[:, kc, h, :],
                                         Eh[:, kc, :],
                                         start=(kc == 0), stop=(kc == 3))
                    nc.scalar.copy(oTr[po:po + 64, h // 2, :], ps4[0:64, :])
                    nc.scalar.copy(sums[0:1, h, :], ps4[64:65, :])

                # ---- normalization matrix R, scale oT ----
                sums6 = wk.tile([6, G], F32, name=f"s6{b}", tag="s6", bufs=1)
                nc.sync.dma_start(sums6[:], sums[0:1, :, :])
                srec = wk.tile([6, G], F32R, name=f"sr{b}", tag="sr", bufs=1)
                with nc.allow_low_precision("fp32r is fp32-width"):
                    nc.vector.reciprocal(srec[:], sums6[:])
                oTs = wk.tile([128, 3, G], BF16, name=f"oTs{b}", tag="oTs",
                              bufs=1)
                for cc in range(3):
                    ps5 = psp.tile([128, G], F32, name=f"R{b}{cc}", tag="ps_a")
                    nc.tensor.matmul(ps5[:],
                                     rep6_s[:, cc * 128:(cc + 1) * 128],
                                     srec[:], start=True, stop=True)
                    nc.vector.tensor_tensor(out=oTs[:, cc, :],
                                            in0=oTr[:, cc, :], in1=ps5[:],
                                            op=OP.mult)

                # ---- proj + residual -> x1 ----
                x1 = wk.tile([128, 4, C], F32, name=f"x1{b}", tag="x1",
                             bufs=2)
                for sch in range(4):
                    ps6 = psp.tile([128, C], F32, name=f"pj{b}{sch}",
                                   tag="ps_b")
                    for cc in range(3):
                        nc.tensor.matmul(ps6[:],
                                         oTs[:, cc, sch * 128:(sch + 1) * 128],
                                         pjT_s[:, cc, :],
                                         start=(cc == 0), stop=(cc == 2))
                    nc.vector.tensor_tensor(out=x1[:, sch, :], in0=ps6[:],
                                            in1=xr[:, sch, :], op=OP.add)

                pb_b = bass.AP(pbr_s.tensor, pbr_s[:].offset,
                               [pbr_s[:].ap[0], [0, 4], pbr_s[:].ap[1]])
                nc.vector.tensor_tensor(out=x1[:], in0=x1[:], in1=pb_b,
                                        op=OP.add)

                # ---- LN2 -> xn2 ----
                xn2 = wk.tile([128, 4, C], BF16, name=f"xn2{b}", tag="xn",
                              bufs=2)
                for ch in range(4):
                    st6b = wk.tile([128, 6], F32, name=f"su{b}{ch}", tag="st")
                    nc.vector.bn_stats(out=st6b[:], in_=x1[:, ch, :])
                    mvb = wk.tile([128, 2], F32, name=f"mw{b}{ch}", tag="mv")
                    nc.vector.bn_aggr(out=mvb[:], in_=st6b[:])
                    sdb = wk.tile([128, 1], F32, name=f"se{b}{ch}", tag="sd")
                    nc.scalar.activation(sdb[:], mvb[:, 1:2], AF.Sqrt,
                                         bias=eps_s[:])
                    rsb = wk.tile([128, 1], F32, name=f"rh{b}{ch}", tag="rg")
                    nc.vector.reciprocal(rsb[:], sdb[:])
                    nc.vector.tensor_scalar(out=xn2[:, ch, :],
                                            in0=x1[:, ch, :],
                                            scalar1=mvb[:, 0:1],
                                            scalar2=rsb[:],
                                            op0=OP.subtract, op1=OP.mult)
                xn2T = wk.tile([128, 3, G], BF16, name=f"x2T{b}", tag="xnT",
                               bufs=2)
                for ch in range(4):
                    eng = nc.sync if ch % 2 == 0 else nc.scalar
                    eng.dma_start_transpose(
                        xn2T[:, :, ch * 128:(ch + 1) * 128], xn2[:, ch, :])

                # ---- fc1 + gelu -> uT ----
                uT = wk.tile([128, 12, G], BF16, name=f"uT{b}", tag="uT",
                             bufs=1)
                for f in range(12):
                    ps7 = psp.tile([128, G], F32, name=f"f1{b}{f}", tag="ps_a")
                    for cc in range(3):
                        nc.tensor.matmul(ps7[:],
                                         w1_s[:, cc, f * 128:(f + 1) * 128],
                                         xn2T[:, cc, :],
                                         start=(cc == 0), stop=(cc == 2))
                    nc.scalar.activation(uT[:, f, :], ps7[:], AF.Gelu,
                                         bias=f1b_s[:, f:f + 1])

                # ---- fc2 + residual -> x2 (bf16, kept in SBUF) ----
                for sch in range(4):
                    ps8 = psp.tile([128, C], F32, name=f"f2{b}{sch}",
                                   tag="ps_b")
                    for f in range(12):
                        nc.tensor.matmul(ps8[:],
                                         uT[:, f, sch * 128:(sch + 1) * 128],
                                         w2_s[:, f, :],
                                         start=(f == 0), stop=(f == 11))
                    x2c = wk.tile([128, C], F32, name=f"x2{b}{sch}",
                                  tag="x2c", bufs=1)
                    nc.vector.tensor_tensor(out=x2c[:], in0=ps8[:],
                                            in1=x1[:, sch, :], op=OP.add)
                    nc.vector.tensor_tensor(out=x2k[b][:, sch, :], in0=x2c[:],
                                            in1=f2br_s[:], op=OP.add)
                    row0 = b * G + sch * 128
                    nc.sync.dma_start(ag_in[row0:row0 + 128, :],
                                      x2k[b][:, sch, :])
                if b % CHB == CHB - 1:
                    j = b // CHB
                    nc.gpsimd.collective_compute(
                        "AllGather", OP.bypass,
                        replica_groups=[list(range(NCORES))],
                        ins=[ag_in[(b - CHB + 1) * G:(b + 1) * G, :]],
                        outs=[table[j * CHROWS:(j + 1) * CHROWS, :]])

            # =================== PHASE C: gather/pool/3NN ===============
            stk.close()
            stk2 = ExitStack()
            gp = stk2.enter_context(tc.tile_pool(name="gat", bufs=2))
            ps2p = stk2.enter_context(
                tc.tile_pool(name="psC", bufs=4, space="PSUM"))
            for b in range(BL):
                acc = gp.tile([128, K + 1, C], BF16, name=f"acc{b}",
                              tag="acc", bufs=2)
                for g in range(5):
                    s0, s1 = g * 8, min(K + 1, (g + 1) * 8)
                    n = (s1 - s0) * 128
                    nc.gpsimd.dma_gather(
                        acc[:, s0:s1, :], table[:, :],
                        gidx_s[:, b, s0 * 8:s1 * 8], n, n, C)
                # max-pool tree over K=32 neighbor slots (in-place halving)
                for half in (16, 8, 4, 2, 1):
                    nc.vector.tensor_tensor(
                        out=acc[:, 0:half, :], in0=acc[:, 0:half, :],
                        in1=acc[:, half:2 * half, :], op=OP.max)
                # BN (x2 & affine folded) + gelu + 0.3*centers, ones col
                pb1 = gp.tile([128, C], F32, name=f"pb1{b}", tag="pb1")
                nc.vector.tensor_tensor(out=pb1[:], in0=acc[:, 0, :],
                                        in1=bns_s[:], op=OP.mult)
                pb2 = gp.tile([128, C], F32, name=f"pb2{b}", tag="pb2")
                nc.vector.tensor_tensor(out=pb2[:], in0=pb1[:], in1=bnh_s[:],
                                        op=OP.add)
                gl = gp.tile([128, C], F32, name=f"gl{b}", tag="gl")
                nc.scalar.activation(gl[:], pb2[:], AF.Gelu)
                visa = gp.tile([128, C + 4], F32R, name=f"vis{b}", tag="vis")
                nc.vector.tensor_copy(visa[:, C:C + 4], ones_s[:])
                nc.vector.scalar_tensor_tensor(
                    out=visa[:, 0:C], in0=acc[:, K, :], scalar=0.3, in1=gl[:],
                    op0=OP.mult, op1=OP.add)

                for ch in range(4):
                    psi = ps2p.tile([128, C + 4], F32, name=f"ip{b}{ch}",
                                    tag="ps_i")
                    nc.tensor.matmul(psi[:], wtsb[b][:, ch, :], visa[:],
                                     start=True, stop=True)
                    rsm = gp.tile([128, 1], F32, name=f"rm{b}{ch}", tag="rm")
                    nc.vector.reciprocal(rsm[:], psi[:, C:C + 1])
                    ocs = gp.tile([128, C], F32, name=f"os{b}{ch}", tag="os")
                    nc.vector.tensor_scalar(out=ocs[:], in0=psi[:, 0:C],
                                            scalar1=rsm[:], scalar2=None,
                                            op0=OP.mult)
                    oc = gp.tile([128, C], F32, name=f"oc{b}{ch}", tag="oc")
                    nc.vector.tensor_tensor(out=oc[:], in0=ocs[:],
                                            in1=x2k[b][:, ch, :], op=OP.add)
                    nc.sync.dma_start(
                        d_out.ap()[b, ch * 128:(ch + 1) * 128, :], oc[:])
            stk2.close()

    nc.compile()
    return nc


def _prep_inputs(x, level1_center, level2_center, ln1_g, ln1_b, qkv_w, proj_w,
                 proj_b, ln2_g, ln2_b, fc1_w, fc1_b, fc2_w, fc2_b, bn_g, bn_b,
                 bn_mean, bn_var, level1_index, level2_index):
    """Build the per-core in_maps (host-side folding + sharding)."""
    f32 = np.float32
    x = np.ascontiguousarray(np.asarray(x, f32))
    l1c = np.asarray(level1_center, f32)
    l2c = np.asarray(level2_center, f32)
    ln1_g = np.asarray(ln1_g, f32); ln1_b = np.asarray(ln1_b, f32)
    ln2_g = np.asarray(ln2_g, f32); ln2_b = np.asarray(ln2_b, f32)
    qkv_w = np.asarray(qkv_w, f32); proj_w = np.asarray(proj_w, f32)
    proj_b = np.asarray(proj_b, f32)
    fc1_w = np.asarray(fc1_w, f32); fc1_b = np.asarray(fc1_b, f32)
    fc2_w = np.asarray(fc2_w, f32); fc2_b = np.asarray(fc2_b, f32)
    bn_g = np.asarray(bn_g, f32); bn_b = np.asarray(bn_b, f32)
    bn_mean = np.asarray(bn_mean, f32); bn_var = np.asarray(bn_var, f32)
    l1i = np.asarray(level1_index).astype(np.int64).reshape(B, N2, K)
    l2i = np.asarray(level2_index).astype(np.int64).reshape(B, N2)

    # remap global row ids to the 4-chunk AllGather table layout:
    # chunk j holds local batches {2j, 2j+1} of every core.
    def _remap(r):
        c = r // (BL * G)
        rem = r % (BL * G)
        b = rem // G
        g = rem % G
        return ((b // CHB) * CHROWS + c * (CHB * G) + (b % CHB) * G + g)

    l1i = _remap(l1i)
    l2i = _remap(l2i)

    s = HD ** -0.5
    weff = qkv_w * ln1_g[None, :]
    beff = qkv_w @ ln1_b
    weff[:C] *= s
    beff[:C] *= s
    wqkT = np.ascontiguousarray(weff[:2 * C].T.astype(ml_dtypes.bfloat16))
    qk_bias = np.ascontiguousarray(beff[:2 * C].reshape(6, 128).T)
    wvT = np.ascontiguousarray(weff[2 * C:].T.astype(ml_dtypes.bfloat16))
    v_bias_rep = np.ascontiguousarray(
        np.broadcast_to(beff[2 * C:], (128, C)))
    projT = np.ascontiguousarray(proj_w.T.astype(ml_dtypes.bfloat16))
    proj_b_rep = np.ascontiguousarray(np.broadcast_to(proj_b, (128, C)))
    w1eff = fc1_w * ln2_g[None, :]
    f1bias = fc1_b + fc1_w @ ln2_b
    wfc1T = np.ascontiguousarray(w1eff.T.astype(ml_dtypes.bfloat16))
    fc1_bias = np.ascontiguousarray(f1bias.reshape(12, 128).T)
    wfc2T = np.ascontiguousarray(fc2_w.T.astype(ml_dtypes.bfloat16))
    fc2_b_rep = np.ascontiguousarray(np.broadcast_to(fc2_b, (128, C)))
    gs = bn_g / np.sqrt(bn_var + EPS_BN)
    bn_scale_rep = np.ascontiguousarray(
        np.broadcast_to((2.0 * gs).astype(f32), (128, C)))
    bn_shift_rep = np.ascontiguousarray(
        np.broadcast_to((bn_b - bn_mean * gs).astype(f32), (128, C)))
    rep6 = np.zeros((6, C), f32)
    for h in range(H):
        rep6[h, h * HD:(h + 1) * HD] = 1.0

    # 3NN augmented coordinate blocks
    l1n = (l1c ** 2).sum(-1)                                 # [B, G]
    l2n = (l2c ** 2).sum(-1)                                 # [B, N2]
    l1aug = np.empty((B, 5, G), f32)
    l1aug[:, 0:3] = np.transpose(l1c, (0, 2, 1))
    l1aug[:, 3] = 1.0
    l1aug[:, 4] = l1n
    l2aug = np.empty((B, 5, N2), f32)
    l2aug[:, 0:3] = -2.0 * np.transpose(l2c, (0, 2, 1))
    l2aug[:, 3] = l2n
    l2aug[:, 4] = 1.0

    shared = {
        "wqkT": wqkT, "qk_bias": qk_bias, "wvT": wvT,
        "v_bias_rep": v_bias_rep, "projT": projT,
        "proj_b_rep": proj_b_rep, "wfc1T": wfc1T, "fc1_bias": fc1_bias,
        "wfc2T": wfc2T, "fc2_b_rep": fc2_b_rep,
        "bn_scale_rep": bn_scale_rep, "bn_shift_rep": bn_shift_rep,
        "rep6": rep6,
    }
    in_maps = []
    for c in range(NCORES):
        b0 = c * BL
        # bulk-gather index tile: [128, BL*ICOLS] int16, wrapped in 16
        # partitions, replicated 8x across partition groups (one per Q7 core).
        gidx = np.empty((128, BL * ICOLS), np.int16)
        for b in range(BL):
            idxs = np.empty((NI,), np.int64)
            # slot j<K at i=j*128+p -> neighbor j of point p; j=K -> center
            idxs[:K * 128] = np.transpose(
                l1i[b0 + b], (1, 0)).reshape(K * 128)
            idxs[K * 128:] = l2i[b0 + b]
            wrap = idxs.reshape(ICOLS, 16).T.astype(np.int16)
            gidx[:, b * ICOLS:(b + 1) * ICOLS] = np.tile(wrap, (8, 1))
        m = dict(shared)
        m["x_in"] = np.ascontiguousarray(x[b0:b0 + BL])
        m["gidx"] = gidx
        m["l1aug"] = np.ascontiguousarray(l1aug[b0:b0 + BL])
        m["l2aug"] = np.ascontiguousarray(l2aug[b0:b0 + BL])
        in_maps.append(m)
    return in_maps


def get_program():
    if "nc" not in _CACHE:
        _CACHE["nc"] = _build_program()
    return _CACHE["nc"]


def run(in_maps, **kw):
    nc = get_program()
    return bass_utils.run_bass_kernel_spmd(
        nc, in_maps, core_ids=list(range(NCORES)), **kw)


def kernel(**inputs):
    in_maps = _prep_inputs(**inputs)
    res = run(in_maps)
    out = np.concatenate([res.results[c]["out"] for c in range(NCORES)],
                         axis=0)
    return out.astype(np.float32)


if __name__ == "__main__":
    np.random.seed(0)
    get_program()
    print("program built + compiled OK")


# revision 4
# speedup vs baseline: 1.2049x; 1.0528x over previous
"""Trainium2 Bass kernel for nn_Block_86921548136965 (gnn_message_passing), v2.

Transformer block (LN->MHA->LN->MLP) + global neighbor max-pool + BN/GELU +
3-NN inverse-distance interpolation, data-parallel over batch across 8
NeuronCores.

v2 changes vs baseline:
- Phase C neighbor gather via one bulk dma_gather per batch (vs 33 SWDGE
  indirect DMAs each) - removes the ~360us serial gpsimd bottleneck.
- 4-chunk AllGather (after local batches 1,3,5,7) so only the last ~3MB
  chunk is exposed.
- 3NN interpolation weights precomputed during Phase A on the otherwise
  idle gpsimd/vector engines.
- x2 kept in SBUF as bf16 (no fp32 DRAM round trip).
- softmax-sum reshape via a single SBUF->SBUF DMA (no DRAM bounce).
- rowsum of interp weights folded into the interp matmul (ones column).
"""
import os
import sys

sys.path.insert(0, "/opt/trn_rl_repo")

import numpy as np
import ml_dtypes

import concourse.bass as bass
import concourse.bacc as bacc
import concourse.tile as tile
from concourse import mybir
from concourse import bass_utils
from concourse.masks import make_identity

# problem shapes
B, G, C, H = 64, 512, 384, 6
HD = C // H  # 64
N2, K = 128, 32
HID = 4 * C  # 1536
NCORES = 8
BL = B // NCORES  # 8 batches per core
ROWS = B * G  # 32768 global rows
CHB = 2  # batches per AllGather chunk
NCHUNK = BL // CHB  # 4
CHROWS = ROWS // NCHUNK  # 8192 rows per chunk
NI = (K + 1) * 128  # 4224 gathered rows per batch (32 neigh + 1 center)
ICOLS = NI // 16  # 264 int16 per partition (16-wrap)

F32 = mybir.dt.float32
F32R = mybir.dt.float32r
BF16 = mybir.dt.bfloat16
I16 = mybir.dt.int16
AX = mybir.AxisListType
OP = mybir.AluOpType
AF = mybir.ActivationFunctionType

BIG = 1.0e30
EPS_LN = 1e-5
EPS_BN = 1e-5
EPS_W = 1e-8

_CACHE = {}


def _build_program():
    nc = bacc.Bacc("TRN2", target_bir_lowering=False, debug=False,
                   num_devices=NCORES, num_swdge_queues=4)

    # ---------------- DRAM I/O ----------------
    d_x = nc.dram_tensor("x_in", [BL, G, C], F32, kind="ExternalInput")
    d_wqk = nc.dram_tensor("wqkT", [C, 2 * C], BF16, kind="ExternalInput")
    d_qkb = nc.dram_tensor("qk_bias", [128, 6], F32, kind="ExternalInput")
    d_wv = nc.dram_tensor("wvT", [C, C], BF16, kind="ExternalInput")
    d_vbr = nc.dram_tensor("v_bias_rep", [128, C], F32, kind="ExternalInput")
    d_pjT = nc.dram_tensor("projT", [C, C], BF16, kind="ExternalInput")
    d_pbr = nc.dram_tensor("proj_b_rep", [128, C], F32, kind="ExternalInput")
    d_w1 = nc.dram_tensor("wfc1T", [C, HID], BF16, kind="ExternalInput")
    d_f1b = nc.dram_tensor("fc1_bias", [128, 12], F32, kind="ExternalInput")
    d_w2 = nc.dram_tensor("wfc2T", [HID, C], BF16, kind="ExternalInput")
    d_f2br = nc.dram_tensor("fc2_b_rep", [128, C], F32, kind="ExternalInput")
    d_bns = nc.dram_tensor("bn_scale_rep", [128, C], F32, kind="ExternalInput")
    d_bnh = nc.dram_tensor("bn_shift_rep", [128, C], F32, kind="ExternalInput")
    d_rep6 = nc.dram_tensor("rep6", [6, C], F32, kind="ExternalInput")
    d_gidx = nc.dram_tensor("gidx", [128, BL * ICOLS], I16,
                            kind="ExternalInput")
    d_l1a = nc.dram_tensor("l1aug", [BL, 5, G], F32, kind="ExternalInput")
    d_l2a = nc.dram_tensor("l2aug", [BL, 5, N2], F32, kind="ExternalInput")
    d_out = nc.dram_tensor("out", [BL, G, C], F32, kind="ExternalOutput")

    from contextlib import ExitStack
    with tile.TileContext(nc) as tc:
        with tc.tile_pool(name="cpool", bufs=1) as cp, \
             tc.tile_pool(name="dram", bufs=1, space="DRAM") as dp:
            stk = ExitStack()
            wp = stk.enter_context(tc.tile_pool(name="wpool", bufs=1))
            wk = stk.enter_context(tc.tile_pool(name="work", bufs=2))
            psp = stk.enter_context(tc.tile_pool(name="ps", bufs=2,
                                                 space="PSUM"))

            # ---------------- static loads ----------------
            wqk_s = wp.tile([128, 3, 2 * C], BF16)
            nc.sync.dma_start(wqk_s[:], d_wqk.ap().rearrange(
                "(cc p) f -> p cc f", p=128))
            wv_s = wp.tile([128, 3, C], BF16)
            nc.sync.dma_start(wv_s[:], d_wv.ap().rearrange(
                "(cc p) f -> p cc f", p=128))
            pjT_s = wp.tile([128, 3, C], BF16)
            nc.sync.dma_start(pjT_s[:], d_pjT.ap().rearrange(
                "(cc p) f -> p cc f", p=128))
            w1_s = wp.tile([128, 3, HID], BF16)
            nc.sync.dma_start(w1_s[:], d_w1.ap().rearrange(
                "(cc p) f -> p cc f", p=128))
            w2_s = wp.tile([128, 12, C], BF16)
            nc.sync.dma_start(w2_s[:], d_w2.ap().rearrange(
                "(cc p) f -> p cc f", p=128))
            rep6_s = wp.tile([6, C], F32R)
            nc.sync.dma_start(rep6_s[:], d_rep6.ap().bitcast(F32R))

            qkb_s = wp.tile([128, 6], F32)
            nc.sync.dma_start(qkb_s[:], d_qkb.ap())
            f1b_s = wp.tile([128, 12], F32)
            nc.sync.dma_start(f1b_s[:], d_f1b.ap())
            vbr_s = wp.tile([128, C], F32)
            nc.sync.dma_start(vbr_s[:], d_vbr.ap())
            pbr_s = wp.tile([128, C], F32)
            nc.sync.dma_start(pbr_s[:], d_pbr.ap())
            f2br_s = wp.tile([128, C], F32)
            nc.sync.dma_start(f2br_s[:], d_f2br.ap())
            bns_s = cp.tile([128, C], F32)
            nc.sync.dma_start(bns_s[:], d_bns.ap())
            bnh_s = cp.tile([128, C], F32)
            nc.sync.dma_start(bnh_s[:], d_bnh.ap())
            gidx_s = cp.tile([128, BL, ICOLS], I16)
            nc.sync.dma_start(gidx_s[:], d_gidx.ap().rearrange(
                "p (b s) -> p b s", b=BL))
            l1a_s = cp.tile([5, BL, G], F32)
            nc.sync.dma_start(l1a_s[:], d_l1a.ap().rearrange("b r s -> r b s"))
            l2a_s = cp.tile([5, BL, N2], F32)
            nc.sync.dma_start(l2a_s[:], d_l2a.ap().rearrange("b r s -> r b s"))

            ident = cp.tile([128, 128], F32)
            make_identity(nc, ident[:])
            eps_s = cp.tile([128, 1], F32)
            nc.vector.memset(eps_s[:], EPS_LN)
            ones_s = cp.tile([128, 4], F32)
            nc.vector.memset(ones_s[:], 1.0)

            # persistent per-batch outputs
            x2k = [cp.tile([128, 4, C], BF16, name=f"x2k{b}")
                   for b in range(BL)]
            wtsb = [cp.tile([128, 4, N2], F32R, name=f"wtsb{b}")
                    for b in range(BL)]

            # internal DRAM
            ag_in = dp.tile([BL * G, C], BF16)
            table = dp.tile([ROWS, C], BF16)

            # =================== PHASE A: transformer ===================
            for b in range(BL):
                xr = wk.tile([128, 4, C], F32, name=f"xr{b}", tag="xr")
                nc.sync.dma_start(xr[:], d_x.ap()[b].rearrange(
                    "(ch p) c -> p ch c", p=128))
                # ---- LN1 -> xn (normalized; affine folded into weights) ----
                xn = wk.tile([128, 4, C], BF16, name=f"xn{b}", tag="xn",
                             bufs=2)
                for ch in range(4):
                    st6 = wk.tile([128, 6], F32, name=f"st{b}{ch}", tag="st")
                    nc.vector.bn_stats(out=st6[:], in_=xr[:, ch, :])
                    mv = wk.tile([128, 2], F32, name=f"mv{b}{ch}", tag="mv")
                    nc.vector.bn_aggr(out=mv[:], in_=st6[:])
                    sd = wk.tile([128, 1], F32, name=f"sd{b}{ch}", tag="sd")
                    nc.scalar.activation(sd[:], mv[:, 1:2], AF.Sqrt,
                                         bias=eps_s[:])
                    rs = wk.tile([128, 1], F32, name=f"rg{b}{ch}", tag="rg")
                    nc.vector.reciprocal(rs[:], sd[:])
                    nc.vector.tensor_scalar(out=xn[:, ch, :], in0=xr[:, ch, :],
                                            scalar1=mv[:, 0:1], scalar2=rs[:],
                                            op0=OP.subtract, op1=OP.mult)
                # ---- transpose xn -> xnT [c, s] ----
                xnT = wk.tile([128, 3, G], BF16, name=f"xnT{b}", tag="xnT",
                              bufs=2)
                for ch in range(4):
                    eng = nc.sync if ch % 2 == 0 else nc.scalar
                    eng.dma_start_transpose(
                        xnT[:, :, ch * 128:(ch + 1) * 128], xn[:, ch, :])

                # ---- qkT = Weff_qk @ xnT + bias ----
                qkT = wk.tile([128, 6, G], BF16, name=f"qkT{b}", tag="qkT",
                              bufs=1)
                for f in range(6):
                    ps1 = psp.tile([128, G], F32, name=f"qk{b}{f}", tag="ps_a")
                    for cc in range(3):
                        nc.tensor.matmul(ps1[:],
                                         wqk_s[:, cc, f * 128:(f + 1) * 128],
                                         xnT[:, cc, :],
                                         start=(cc == 0), stop=(cc == 2))
                    nc.vector.tensor_scalar(out=qkT[:, f, :], in0=ps1[:],
                                            scalar1=qkb_s[:, f:f + 1],
                                            scalar2=None, op0=OP.add)

                # ---- v = xn @ WvT + bias, stored as vaug [s, h, 65] ----
                vaug = wk.tile([128, 4, 6, 65], BF16, name=f"va{b}", tag="va",
                               bufs=1)
                nc.vector.memset(vaug[:, :, :, 64:65], 1.0)
                for sch in range(4):
                    ps2 = psp.tile([128, C], F32, name=f"v{b}{sch}",
                                   tag="ps_b")
                    for cc in range(3):
                        nc.tensor.matmul(ps2[:],
                                         xnT[:, cc, sch * 128:(sch + 1) * 128],
                                         wv_s[:, cc, :],
                                         start=(cc == 0), stop=(cc == 2))
                    nc.vector.tensor_tensor(
                        out=vaug[:, sch, :, 0:64],
                        in0=ps2[:].rearrange("p (h d) -> p h d", h=6),
                        in1=vbr_s[:].rearrange("p (h d) -> p h d", h=6),
                        op=OP.add)

                # ---- 3NN interpolation weights for batch b ----
                # Stage-interleaved across the 4 row-chunks so the
                # vector<->gpsimd ping-pong overlaps instead of serializing.
                # d2 = l1aug^T @ l2aug; top-3 min mask; w = mask/(d2+eps);
                # transposed un-normalized weights to wtsb (rowsum comes from
                # the ones column of vis_aug in phase C).
                d2s_t, m1_t, k1_t, da_t, m2_t = [], [], [], [], []
                k2_t, db_t, m3_t, kk_t, de_t, wi_t, w0_t = [], [], [], [], [], [], []
                for ch in range(4):
                    psd = psp.tile([128, N2], F32, name=f"d2{b}{ch}",
                                   tag="ps_c")
                    nc.tensor.matmul(psd[:],
                                     l1a_s[:, b, ch * 128:(ch + 1) * 128],
                                     l2a_s[:, b, :], start=True, stop=True)
                    d2s = wk.tile([128, N2], F32, name=f"d2s{b}{ch}",
                                  tag=f"d2s{ch}", bufs=1)
                    nc.scalar.copy(d2s[:], psd[:])
                    d2s_t.append(d2s)
                for ch in range(4):
                    m1 = wk.tile([128, 1], F32, name=f"m1{b}{ch}",
                                 tag=f"m1{ch}", bufs=1)
                    nc.vector.tensor_reduce(out=m1[:], in_=d2s_t[ch][:],
                                            axis=AX.X, op=OP.min)
                    m1_t.append(m1)
                for ch in range(4):
                    msk1 = wk.tile([128, N2], F32, name=f"k1{b}{ch}",
                                   tag=f"k1{ch}", bufs=1)
                    nc.vector.tensor_scalar(out=msk1[:], in0=d2s_t[ch][:],
                                            scalar1=m1_t[ch][:], scalar2=BIG,
                                            op0=OP.is_le, op1=OP.mult)
                    k1_t.append(msk1)
                for ch in range(4):
                    d2a = wk.tile([128, N2], F32, name=f"da{b}{ch}",
                                  tag=f"da{ch}", bufs=1)
                    nc.vector.tensor_tensor(out=d2a[:], in0=d2s_t[ch][:],
                                            in1=k1_t[ch][:], op=OP.add)
                    da_t.append(d2a)
                for ch in range(4):
                    m2 = wk.tile([128, 1], F32, name=f"m2{b}{ch}",
                                 tag=f"m2{ch}", bufs=1)
                    nc.vector.tensor_reduce(out=m2[:], in_=da_t[ch][:],
                                            axis=AX.X, op=OP.min)
                    m2_t.append(m2)
                for ch in range(4):
                    msk2 = wk.tile([128, N2], F32, name=f"k2{b}{ch}",
                                   tag=f"k1{ch}", bufs=1)
                    nc.vector.tensor_scalar(out=msk2[:], in0=da_t[ch][:],
                                            scalar1=m2_t[ch][:], scalar2=BIG,
                                            op0=OP.is_le, op1=OP.mult)
                    k2_t.append(msk2)
                for ch in range(4):
                    nc.vector.tensor_tensor(out=da_t[ch][:], in0=da_t[ch][:],
                                            in1=k2_t[ch][:], op=OP.add)
                    db_t.append(da_t[ch])
                for ch in range(4):
                    m3 = wk.tile([128, 1], F32, name=f"m3{b}{ch}",
                                 tag=f"m3{ch}", bufs=1)
                    nc.vector.tensor_reduce(out=m3[:], in_=db_t[ch][:],
                                            axis=AX.X, op=OP.min)
                    m3_t.append(m3)
                for ch in range(4):
                    msk = wk.tile([128, N2], F32, name=f"kk{b}{ch}",
                                  tag=f"k1{ch}", bufs=1)
                    nc.vector.tensor_scalar(out=msk[:], in0=d2s_t[ch][:],
                                            scalar1=m3_t[ch][:], scalar2=None,
                                            op0=OP.is_le)
                    kk_t.append(msk)
                    d2e = wk.tile([128, N2], F32, name=f"de{b}{ch}",
                                  tag=f"da{ch}", bufs=1)
                    nc.vector.tensor_scalar(out=d2e[:], in0=d2s_t[ch][:],
                                            scalar1=EPS_W, scalar2=None,
                                            op0=OP.add)
                    de_t.append(d2e)
                for ch in range(4):
                    wiv = wk.tile([128, N2], F32, name=f"wi{b}{ch}",
                                  tag=f"wi{ch}", bufs=1)
                    nc.vector.reciprocal(wiv[:], de_t[ch][:])
                    wi_t.append(wiv)
                for ch in range(4):
                    w0 = wk.tile([128, N2], F32, name=f"w0{b}{ch}",
                                 tag=f"w0{ch}", bufs=1)
                    nc.vector.tensor_tensor(out=w0[:], in0=kk_t[ch][:],
                                            in1=wi_t[ch][:], op=OP.mult)
                    w0_t.append(w0)
                for ch in range(4):
                    pst = psp.tile([128, N2], F32, name=f"wt{b}{ch}",
                                   tag="ps_c")
                    nc.tensor.transpose(pst[:], w0_t[ch][:], ident[:])
                    nc.vector.tensor_copy(wtsb[b][:, ch, :], pst[:])

                # ---- attention per head ----
                oTr = wk.tile([128, 3, G], BF16, name=f"oTr{b}", tag="oTr",
                              bufs=2)
                sums = wk.tile([1, 6, G], F32, name=f"sm{b}", tag="sm",
                               bufs=1)
                for h in range(6):
                    po = (h % 2) * 64
                    qT = qkT[po:po + 64, h // 2, :]
                    kT = qkT[po:po + 64, 3 + h // 2, :]
                    Eh = wk.tile([128, 4, G], BF16, name=f"E{b}{h}", tag="E",
                                 bufs=3)
                    for kc in range(4):
                        ps3 = psp.tile([128, G], F32, name=f"s{b}{h}{kc}",
                                       tag="ps_a")
                        nc.tensor.matmul(ps3[:],
                                         kT[:, kc * 128:(kc + 1) * 128],
                                         qT, start=True, stop=True)
                        nc.scalar.activation(Eh[:, kc, :], ps3[:], AF.Exp)
                    ps4 = psp.tile([65, G], F32, name=f"o{b}{h}", tag="ps_c")
                    for kc in range(4):
                        nc.tensor.matmul(ps4[:], vaug[:, kc, h, :],
                                         Eh[:, kc, :],
                                         start=(kc == 0), stop=(kc == 3))
                    nc.scalar.copy(oTr[po:po + 64, h // 2, :], ps4[0:64, :])
                    nc.scalar.copy(sums[0:1, h, :], ps4[64:65, :])

                # ---- normalization matrix R, scale oT ----
                sums6 = wk.tile([6, G], F32, name=f"s6{b}", tag="s6", bufs=1)
                nc.sync.dma_start(sums6[:], sums[0:1, :, :])
                srec = wk.tile([6, G], F32R, name=f"sr{b}", tag="sr", bufs=1)
                with nc.allow_low_precision("fp32r is fp32-width"):
                    nc.vector.reciprocal(srec[:], sums6[:])
                oTs = wk.tile([128, 3, G], BF16, name=f"oTs{b}", tag="oTs",
                              bufs=1)
                for cc in range(3):
                    ps5 = psp.tile([128, G], F32, name=f"R{b}{cc}", tag="ps_a")
                    nc.tensor.matmul(ps5[:],
                                     rep6_s[:, cc * 128:(cc + 1) * 128],
                                     srec[:], start=True, stop=True)
                    nc.vector.tensor_tensor(out=oTs[:, cc, :],
                                            in0=oTr[:, cc, :], in1=ps5[:],
                                            op=OP.mult)

                # ---- proj + residual -> x1 ----
                x1 = wk.tile([128, 4, C], F32, name=f"x1{b}", tag="x1",
                             bufs=2)
                for sch in range(4):
                    ps6 = psp.tile([128, C], F32, name=f"pj{b}{sch}",
                                   tag="ps_b")
                    for cc in range(3):
                        nc.tensor.matmul(ps6[:],
                                         oTs[:, cc, sch * 128:(sch + 1) * 128],
                                         pjT_s[:, cc, :],
                                         start=(cc == 0), stop=(cc == 2))
                    nc.vector.tensor_tensor(out=x1[:, sch, :], in0=ps6[:],
                                            in1=xr[:, sch, :], op=OP.add)

                pb_b = bass.AP(pbr_s.tensor, pbr_s[:].offset,
                               [pbr_s[:].ap[0], [0, 4], pbr_s[:].ap[1]])
                nc.vector.tensor_tensor(out=x1[:], in0=x1[:], in1=pb_b,
                                        op=OP.add)

                # ---- LN2 -> xn2 ----
                xn2 = wk.tile([128, 4, C], BF16, name=f"xn2{b}", tag="xn",
                              bufs=2)
                for ch in range(4):
                    st6b = wk.tile([128, 6], F32, name=f"su{b}{ch}", tag="st")
                    nc.vector.bn_stats(out=st6b[:], in_=x1[:, ch, :])
                    mvb = wk.tile([128, 2], F32, name=f"mw{b}{ch}", tag="mv")
                    nc.vector.bn_aggr(out=mvb[:], in_=st6b[:])
                    sdb = wk.tile([128, 1], F32, name=f"se{b}{ch}", tag="sd")
                    nc.scalar.activation(sdb[:], mvb[:, 1:2], AF.Sqrt,
                                         bias=eps_s[:])
                    rsb = wk.tile([128, 1], F32, name=f"rh{b}{ch}", tag="rg")
                    nc.vector.reciprocal(rsb[:], sdb[:])
                    nc.vector.tensor_scalar(out=xn2[:, ch, :],
                                            in0=x1[:, ch, :],
                                            scalar1=mvb[:, 0:1],
                                            scalar2=rsb[:],
                                            op0=OP.subtract, op1=OP.mult)
                xn2T = wk.tile([128, 3, G], BF16, name=f"x2T{b}", tag="xnT",
                               bufs=2)
                for ch in range(4):
                    eng = nc.sync if ch % 2 == 0 else nc.scalar
                    eng.dma_start_transpose(
                        xn2T[:, :, ch * 128:(ch + 1) * 128], xn2[:, ch, :])

                # ---- fc1 + gelu -> uT ----
                uT = wk.tile([128, 12, G], BF16, name=f"uT{b}", tag="uT",
                             bufs=1)
                for f in range(12):
                    ps7 = psp.tile([128, G], F32, name=f"f1{b}{f}", tag="ps_a")
                    for cc in range(3):
                        nc.tensor.matmul(ps7[:],
                                         w1_s[:, cc, f * 128:(f + 1) * 128],
                                         xn2T[:, cc, :],
                                         start=(cc == 0), stop=(cc == 2))
                    nc.scalar.activation(uT[:, f, :], ps7[:], AF.Gelu,
                                         bias=f1b_s[:, f:f + 1])

                # ---- fc2 + residual -> x2 (bf16, kept in SBUF) ----
                for sch in range(4):
                    ps8 = psp.tile([128, C], F32, name=f"f2{b}{sch}",
                                   tag="ps_b")
                    for f in range(12):
                        nc.tensor.matmul(ps8[:],
                                         uT[:, f, sch * 128:(sch + 1) * 128],
                                         w2_s[:, f, :],
                                         start=(f == 0), stop=(f == 11))
                    x2c = wk.tile([128, C], F32, name=f"x2{b}{sch}",
                                  tag="x2c", bufs=1)
                    nc.vector.tensor_tensor(out=x2c[:], in0=ps8[:],
                                            in1=x1[:, sch, :], op=OP.add)
                    nc.vector.tensor_tensor(out=x2k[b][:, sch, :], in0=x2c[:],
                                            in1=f2br_s[:], op=OP.add)
                    row0 = b * G + sch * 128
                    nc.sync.dma_start(ag_in[row0:row0 + 128, :],
                                      x2k[b][:, sch, :])
                if b % CHB == CHB - 1:
                    j = b // CHB
                    nc.gpsimd.collective_compute(
                        "AllGather", OP.bypass,
                        replica_groups=[list(range(NCORES))],
                        ins=[ag_in[(b - CHB + 1) * G:(b + 1) * G, :]],
                        outs=[table[j * CHROWS:(j + 1) * CHROWS, :]])

            # =================== PHASE C: gather/pool/3NN ===============
            stk.close()
            stk2 = ExitStack()
            gp = stk2.enter_context(tc.tile_pool(name="gat", bufs=2))
            ps2p = stk2.enter_context(
                tc.tile_pool(name="psC", bufs=4, space="PSUM"))
            for b in range(BL):
                acc = gp.tile([128, K + 1, C], BF16, name=f"acc{b}",
                              tag="acc", bufs=2)
                for g in range(5):
                    s0, s1 = g * 8, min(K + 1, (g + 1) * 8)
                    n = (s1 - s0) * 128
                    nc.gpsimd.dma_gather(
                        acc[:, s0:s1, :], table[:, :],
                        gidx_s[:, b, s0 * 8:s1 * 8], n, n, C,
                        queue_num=(b * 5 + g) % 4)
                # max-pool tree over K=32 neighbor slots (in-place halving)
                for half in (16, 8, 4, 2, 1):
                    nc.vector.tensor_tensor(
                        out=acc[:, 0:half, :], in0=acc[:, 0:half, :],
                        in1=acc[:, half:2 * half, :], op=OP.max)
                # BN (x2 & affine folded) + gelu + 0.3*centers, ones col
                pb1 = gp.tile([128, C], F32, name=f"pb1{b}", tag="pb1")
                nc.vector.tensor_tensor(out=pb1[:], in0=acc[:, 0, :],
                                        in1=bns_s[:], op=OP.mult)
                pb2 = gp.tile([128, C], F32, name=f"pb2{b}", tag="pb2")
                nc.vector.tensor_tensor(out=pb2[:], in0=pb1[:], in1=bnh_s[:],
                                        op=OP.add)
                gl = gp.tile([128, C], F32, name=f"gl{b}", tag="gl")
                nc.scalar.activation(gl[:], pb2[:], AF.Gelu)
                visa = gp.tile([128, C + 4], F32R, name=f"vis{b}", tag="vis")
                nc.vector.tensor_copy(visa[:, C:C + 4], ones_s[:])
                nc.vector.scalar_tensor_tensor(
                    out=visa[:, 0:C], in0=acc[:, K, :], scalar=0.3, in1=gl[:],
                    op0=OP.mult, op1=OP.add)

                for ch in range(4):
                    psi = ps2p.tile([128, C + 4], F32, name=f"ip{b}{ch}",
                                    tag="ps_i")
                    nc.tensor.matmul(psi[:], wtsb[b][:, ch, :], visa[:],
                                     start=True, stop=True)
                    rsm = gp.tile([128, 1], F32, name=f"rm{b}{ch}", tag="rm")
                    nc.vector.reciprocal(rsm[:], psi[:, C:C + 1])
                    ocs = gp.tile([128, C], F32, name=f"os{b}{ch}", tag="os")
                    nc.vector.tensor_scalar(out=ocs[:], in0=psi[:, 0:C],
                                            scalar1=rsm[:], scalar2=None,
                                            op0=OP.mult)
                    oc = gp.tile([128, C], F32, name=f"oc{b}{ch}", tag="oc")
                    nc.vector.tensor_tensor(out=oc[:], in0=ocs[:],
                                            in1=x2k[b][:, ch, :], op=OP.add)
                    nc.sync.dma_start(
                        d_out.ap()[b, ch * 128:(ch + 1) * 128, :], oc[:])
            stk2.close()

    nc.compile()
    return nc


def _prep_inputs(x, level1_center, level2_center, ln1_g, ln1_b, qkv_w, proj_w,
                 proj_b, ln2_g, ln2_b, fc1_w, fc1_b, fc2_w, fc2_b, bn_g, bn_b,
                 bn_mean, bn_var, level1_index, level2_index):
    """Build the per-core in_maps (host-side folding + sharding)."""
    f32 = np.float32
    x = np.ascontiguousarray(np.asarray(x, f32))
    l1c = np.asarray(level1_center, f32)
    l2c = np.asarray(level2_center, f32)
    ln1_g = np.asarray(ln1_g, f32); ln1_b = np.asarray(ln1_b, f32)
    ln2_g = np.asarray(ln2_g, f32); ln2_b = np.asarray(ln2_b, f32)
    qkv_w = np.asarray(qkv_w, f32); proj_w = np.asarray(proj_w, f32)
    proj_b = np.asarray(proj_b, f32)
    fc1_w = np.asarray(fc1_w, f32); fc1_b = np.asarray(fc1_b, f32)
    fc2_w = np.asarray(fc2_w, f32); fc2_b = np.asarray(fc2_b, f32)
    bn_g = np.asarray(bn_g, f32); bn_b = np.asarray(bn_b, f32)
    bn_mean = np.asarray(bn_mean, f32); bn_var = np.asarray(bn_var, f32)
    l1i = np.asarray(level1_index).astype(np.int64).reshape(B, N2, K)
    l2i = np.asarray(level2_index).astype(np.int64).reshape(B, N2)

    # remap global row ids to the 4-chunk AllGather table layout:
    # chunk j holds local batches {2j, 2j+1} of every core.
    def _remap(r):
        c = r // (BL * G)
        rem = r % (BL * G)
        b = rem // G
        g = rem % G
        return ((b // CHB) * CHROWS + c * (CHB * G) + (b % CHB) * G + g)

    l1i = _remap(l1i)
    l2i = _remap(l2i)

    s = HD ** -0.5
    weff = qkv_w * ln1_g[None, :]
    beff = qkv_w @ ln1_b
    weff[:C] *= s
    beff[:C] *= s
    wqkT = np.ascontiguousarray(weff[:2 * C].T.astype(ml_dtypes.bfloat16))
    qk_bias = np.ascontiguousarray(beff[:2 * C].reshape(6, 128).T)
    wvT = np.ascontiguousarray(weff[2 * C:].T.astype(ml_dtypes.bfloat16))
    v_bias_rep = np.ascontiguousarray(
        np.broadcast_to(beff[2 * C:], (128, C)))
    projT = np.ascontiguousarray(proj_w.T.astype(ml_dtypes.bfloat16))
    proj_b_rep = np.ascontiguousarray(np.broadcast_to(proj_b, (128, C)))
    w1eff = fc1_w * ln2_g[None, :]
    f1bias = fc1_b + fc1_w @ ln2_b
    wfc1T = np.ascontiguousarray(w1eff.T.astype(ml_dtypes.bfloat16))
    fc1_bias = np.ascontiguousarray(f1bias.reshape(12, 128).T)
    wfc2T = np.ascontiguousarray(fc2_w.T.astype(ml_dtypes.bfloat16))
    fc2_b_rep = np.ascontiguousarray(np.broadcast_to(fc2_b, (128, C)))
    gs = bn_g / np.sqrt(bn_var + EPS_BN)
    bn_scale_rep = np.ascontiguousarray(
        np.broadcast_to((2.0 * gs).astype(f32), (128, C)))
    bn_shift_rep = np.ascontiguousarray(
        np.broadcast_to((bn_b - bn_mean * gs).astype(f32), (128, C)))
    rep6 = np.zeros((6, C), f32)
    for h in range(H):
        rep6[h, h * HD:(h + 1) * HD] = 1.0

    # 3NN augmented coordinate blocks
    l1n = (l1c ** 2).sum(-1)                                 # [B, G]
    l2n = (l2c ** 2).sum(-1)                                 # [B, N2]
    l1aug = np.empty((B, 5, G), f32)
    l1aug[:, 0:3] = np.transpose(l1c, (0, 2, 1))
    l1aug[:, 3] = 1.0
    l1aug[:, 4] = l1n
    l2aug = np.empty((B, 5, N2), f32)
    l2aug[:, 0:3] = -2.0 * np.transpose(l2c, (0, 2, 1))
    l2aug[:, 3] = l2n
    l2aug[:, 4] = 1.0

    shared = {
        "wqkT": wqkT, "qk_bias": qk_bias, "wvT": wvT,
        "v_bias_rep": v_bias_rep, "projT": projT,
        "proj_b_rep": proj_b_rep, "wfc1T": wfc1T, "fc1_bias": fc1_bias,
        "wfc2T": wfc2T, "fc2_b_rep": fc2_b_rep,
        "bn_scale_rep": bn_scale_rep, "bn_shift_rep": bn_shift_rep,
        "rep6": rep6,
    }
    in_maps = []
    for c in range(NCORES):
        b0 = c * BL
        # bulk-gather index tile: [128, BL*ICOLS] int16, wrapped in 16
        # partitions, replicated 8x across partition groups (one per Q7 core).
        gidx = np.empty((128, BL * ICOLS), np.int16)
        for b in range(BL):
            idxs = np.empty((NI,), np.int64)
            # slot j<K at i=j*128+p -> neighbor j of point p; j=K -> center
            idxs[:K * 128] = np.transpose(
                l1i[b0 + b], (1, 0)).reshape(K * 128)
            idxs[K * 128:] = l2i[b0 + b]
            wrap = idxs.reshape(ICOLS, 16).T.astype(np.int16)
            gidx[:, b * ICOLS:(b + 1) * ICOLS] = np.tile(wrap, (8, 1))
        m = dict(shared)
        m["x_in"] = np.ascontiguousarray(x[b0:b0 + BL])
        m["gidx"] = gidx
        m["l1aug"] = np.ascontiguousarray(l1aug[b0:b0 + BL])
        m["l2aug"] = np.ascontiguousarray(l2aug[b0:b0 + BL])
        in_maps.append(m)
    return in_maps


def get_program():
    if "nc" not in _CACHE:
        _CACHE["nc"] = _build_program()
    return _CACHE["nc"]


def run(in_maps, **kw):
    nc = get_program()
    return bass_utils.run_bass_kernel_spmd(
        nc, in_maps, core_ids=list(range(NCORES)), **kw)


def kernel(**inputs):
    in_maps = _prep_inputs(**inputs)
    res = run(in_maps)
    out = np.concatenate([res.results[c]["out"] for c in range(NCORES)],
                         axis=0)
    return out.astype(np.float32)


if __name__ == "__main__":
    np.random.seed(0)
    get_program()
    print("program built + compiled OK")
